# revision 1
# baseline (speedup 1.0000x reference)
"""Distributed GraphSAGE kernel for Trainium2 (8 NeuronCores, Bass/Tile).

Takes FULL inputs (same keys as setup_inputs()), shards by graph id across 8
cores, runs a single SPMD Bass program (3 SAGE layers + global mean pool +
linear head) with inter-layer AllGathers, returns the FULL [512, 2] output.

Algorithm per core (nodes sharded by graph; batch is sorted so each core owns
a contiguous node range; edges assigned to the core owning their dst):
  - table TBL_l holds m_l = h_{l-1} @ Wl_l for ALL nodes (fp16, allgathered);
    TBL is split into 4 row-chunks of 2*n_own rows so int16 dma_gather
    indices can address it
  - mean-aggregation for a 128-node tile: dma_gather m_l[src] rows for the
    tile's edge slots (static per-(tile,chunk) budget te_c), build one-hot
    masks (iota == dst_rel) * (1/deg[dst]) on DVE, reduce with TensorE
    matmuls into PSUM -> (mean_agg @ Wl_l)^T directly
  - self term Wr_l^T @ h_{l-1}^T accumulates into the same PSUM
  - relu+bias on ACT; write h_l^T slab (local) and m_{l+1} slab -> AllGather
  - layer 3: transpose h3 to row-major, mask-matmul pooling by graph id with
    1/graph_size folded into the mask, then Wlin matmul + bias
"""
import sys
import os

sys.path.insert(0, "/opt/trn_rl_repo")

import numpy as np
from contextlib import ExitStack
from dataclasses import dataclass

from concourse import bass, mybir, tile, bacc
from concourse import bass_utils
from concourse.masks import make_identity

P = 128
CH = 4              # table row chunks (int16 index limit)
F16 = mybir.dt.float16
F32 = mybir.dt.float32
I16 = mybir.dt.int16

DST_SENTINEL = -1.0e6

# ablation switches for perf diagnosis (set before build_program):
# "noag", "nogather", "nomaskmm", "nomask", "noselfdma", "nobarrier"
ABLATE = frozenset()


@dataclass(frozen=True)
class Cfg:
    n_cores: int
    num_nodes: int
    num_edges: int
    in_feat: int
    hidden: int
    num_graphs: int
    num_classes: int
    n_own: int           # padded nodes per core (multiple of NB*128)
    te_c: tuple          # edge sub-tiles (of 128 slots) per node tile, per chunk
    nb: int              # node tiles per gather batch
    gpc: int             # graphs per core

    @property
    def nt(self):
        return self.n_own // P

    @property
    def te(self):
        return sum(self.te_c)

    @property
    def nte(self):
        return self.nt * self.te


def build_program(cfg: Cfg):
    """Build the SPMD Bass program. Returns compiled nc."""
    nc = bacc.Bacc(
        "TRN2",
        target_bir_lowering=False,
        debug=False,
        num_devices=cfg.n_cores,
        num_swdge_queues=1,
    )

    NT, NB, NTE = cfg.nt, cfg.nb, cfg.nte
    TE = cfg.te
    TEC = cfg.te_c
    OFFC = [sum(TEC[:c]) for c in range(CH)]
    HID = cfg.hidden
    INF = cfg.in_feat
    GPC = cfg.gpc
    NC = cfg.n_cores
    CHROWS = 2 * cfg.n_own

    # ---- I/O -------------------------------------------------------------
    xT_d = nc.dram_tensor("xT", [INF, cfg.n_own], F16, kind="ExternalInput")
    idx_d = nc.dram_tensor("eidx", [P, NTE * 8], I16, kind="ExternalInput")
    dstrel_d = nc.dram_tensor("edstrel", [P, NTE], F32, kind="ExternalInput")
    w_d = nc.dram_tensor("ew", [P, NTE], F32, kind="ExternalInput")
    br_d = nc.dram_tensor("brel", [P, NT], F32, kind="ExternalInput")
    gw_d = nc.dram_tensor("gw", [P, NT], F32, kind="ExternalInput")
    Wl1_d = nc.dram_tensor("Wl1", [INF, HID], F16, kind="ExternalInput")
    Wr1_d = nc.dram_tensor("Wr1", [INF, HID], F16, kind="ExternalInput")
    Wl2_d = nc.dram_tensor("Wl2", [HID, HID], F16, kind="ExternalInput")
    Wr2_d = nc.dram_tensor("Wr2", [HID, HID], F16, kind="ExternalInput")
    Wl3_d = nc.dram_tensor("Wl3", [HID, HID], F16, kind="ExternalInput")
    Wr3_d = nc.dram_tensor("Wr3", [HID, HID], F16, kind="ExternalInput")
    Wlin_d = nc.dram_tensor("Wlin", [HID, cfg.num_classes], F16, kind="ExternalInput")
    bl1_d = nc.dram_tensor("bl1", [HID, 1], F32, kind="ExternalInput")
    bl2_d = nc.dram_tensor("bl2", [HID, 1], F32, kind="ExternalInput")
    bl3_d = nc.dram_tensor("bl3", [HID, 1], F32, kind="ExternalInput")
    blin_d = nc.dram_tensor("blin", [cfg.num_classes, 1], F32, kind="ExternalInput")
    out_d = nc.dram_tensor("out", [cfg.num_classes, GPC], F32, kind="ExternalOutput")

    rg = [list(range(NC))]

    with tile.TileContext(nc) as tc, ExitStack() as ctx:
        sb = ctx.enter_context(tc.tile_pool(name="sb", bufs=1))
        sb2 = ctx.enter_context(tc.tile_pool(name="sb2", bufs=4))
        gbuf = ctx.enter_context(tc.tile_pool(name="gbuf", bufs=2))
        ps = ctx.enter_context(tc.tile_pool(name="ps", bufs=2, space="PSUM"))
        pool_ps = ctx.enter_context(tc.tile_pool(name="pps", bufs=1, space="PSUM"))
        dram = ctx.enter_context(tc.tile_pool(name="dram", bufs=1, space="DRAM"))

        # ---- static SBUF state ------------------------------------------
        iota_i = sb.tile([P, P], mybir.dt.int32)
        nc.gpsimd.iota(iota_i[:], pattern=[[1, P]], base=0, channel_multiplier=0)
        iota_f = sb.tile([P, P], F32)
        nc.vector.tensor_copy(iota_f[:], iota_i[:])
        ident16 = sb.tile([P, P], F16)
        make_identity(nc, ident16[:])

        idx_sb = sb.tile([P, NTE * 8], I16)
        nc.sync.dma_start(idx_sb[:], idx_d[:, :])
        dstrel_sb = sb.tile([P, NTE], F32)
        nc.sync.dma_start(dstrel_sb[:], dstrel_d[:, :])
        w_sb = sb.tile([P, NTE], F32)
        nc.sync.dma_start(w_sb[:], w_d[:, :])
        br_sb = sb.tile([P, NT], F32)
        nc.sync.dma_start(br_sb[:], br_d[:, :])
        gw_sb = sb.tile([P, NT], F32)
        nc.sync.dma_start(gw_sb[:], gw_d[:, :])

        def load_w(d, p_, f_, nm):
            t = sb.tile([p_, f_], F16, name=nm, tag=nm)
            nc.sync.dma_start(t[:], d[:, :])
            return t

        Wl1_sb = load_w(Wl1_d, INF, HID, "wl1s")
        Wr1_sb = load_w(Wr1_d, INF, HID, "wr1s")
        Wl2_sb = load_w(Wl2_d, HID, HID, "wl2s")
        Wr2_sb = load_w(Wr2_d, HID, HID, "wr2s")
        Wl3_sb = load_w(Wl3_d, HID, HID, "wl3s")
        Wr3_sb = load_w(Wr3_d, HID, HID, "wr3s")
        Wlin_sb = load_w(Wlin_d, HID, cfg.num_classes, "wlins")
        bl1_sb = sb.tile([HID, 1], F32)
        nc.sync.dma_start(bl1_sb[:], bl1_d[:, :])
        bl2_sb = sb.tile([HID, 1], F32)
        nc.sync.dma_start(bl2_sb[:], bl2_d[:, :])
        bl3_sb = sb.tile([HID, 1], F32)
        nc.sync.dma_start(bl3_sb[:], bl3_d[:, :])
        blin_sb = sb.tile([cfg.num_classes, 1], F32)
        nc.sync.dma_start(blin_sb[:], blin_d[:, :])

        xT_sb = sb.tile([INF, cfg.n_own], F16)
        nc.sync.dma_start(xT_sb[:], xT_d[:, :])
        tc.no_sync_barrier()

        # ---- internal DRAM ----------------------------------------------
        slabs = [dram.tile([cfg.n_own, HID], F16, tag=f"slab{l}", name=f"slab{l}")
                 for l in range(3)]
        tbls = [dram.tile([NC * cfg.n_own, HID], F16, tag=f"tbl{l}", name=f"tbl{l}")
                for l in range(3)]
        hts = [dram.tile([HID, cfg.n_own], F16, tag=f"ht{l}", name=f"ht{l}")
               for l in range(2)]

        # ---- P0: m1 = x @ Wl1 (row-major slab) --------------------------
        for t in range(NT):
            m_ps = ps.tile([P, HID], F32, tag="mps")
            nc.tensor.matmul(
                out=m_ps[:], lhsT=xT_sb[:, t * P:(t + 1) * P], rhs=Wl1_sb[:],
                start=True, stop=True,
            )
            m_sb = sb2.tile([P, HID], F16, tag="msb")
            nc.vector.tensor_copy(m_sb[:], m_ps[:])
            nc.sync.dma_start(slabs[0][t * P:(t + 1) * P, :], m_sb[:])

        if "noag" not in ABLATE:
            nc.gpsimd.collective_compute(
                "AllGather", mybir.AluOpType.bypass, replica_groups=rg,
                ins=[slabs[0].opt()], outs=[tbls[0].opt()],
            )
        tc.no_sync_barrier()

        # ---- layers ------------------------------------------------------
        for layer in range(3):
            tbl = tbls[layer]
            Wr_sb = (Wr1_sb, Wr2_sb, Wr3_sb)[layer]
            bl_sb = (bl1_sb, bl2_sb, bl3_sb)[layer]
            Wl_next = (Wl2_sb, Wl3_sb, None)[layer]

            if layer == 2:
                poolT_ps = pool_ps.tile([HID, GPC], F32, tag="pool")

            for b in range(NT // NB):
                g_t = gbuf.tile([P, NB * TE * P], F16, tag="g")
                for c in range(CH):
                    if TEC[c] == 0:
                        continue
                    ncols = NB * TEC[c]           # edge sub-tiles in this call
                    col0 = b * NB * TE + OFFC[c] * NB   # global col of call start
                    rel0 = OFFC[c] * NB                 # col within g_t
                    nidx = ncols * P
                    if "nogather" in ABLATE:
                        continue
                    if "seqgather" in ABLATE:
                        nc.sync.dma_start(
                            g_t[:, rel0 * P:(rel0 + ncols) * P],
                            tbl[c * CHROWS:c * CHROWS + nidx, :].rearrange(
                                "(p a) e -> p (a e)", p=P),
                        )
                        continue
                    nc.gpsimd.dma_gather(
                        out_ap=g_t[:, rel0 * P:(rel0 + ncols) * P].rearrange(
                            "p (t e) -> p t e", e=HID),
                        in_ap=tbl[c * CHROWS:(c + 1) * CHROWS, :],
                        idxs_ap=idx_sb[:, col0 * 8:(col0 + ncols) * 8],
                        num_idxs=nidx,
                        num_idxs_reg=nidx,
                        elem_size=HID,
                        single_packet=False,
                    )
                for ti in range(NB):
                    t = b * NB + ti
                    out_ps = ps.tile([HID, P], F32, tag="outT")
                    first = True
                    for c in range(CH):
                        if "nomaskmm" in ABLATE or "nogather" in ABLATE:
                            continue
                        for j in range(TEC[c]):
                            k_rel = OFFC[c] * NB + ti * TEC[c] + j
                            k = b * NB * TE + k_rel
                            mask = sb2.tile([P, P], F16, tag="mask")
                            nc.vector.tensor_scalar(
                                out=mask[:], in0=iota_f[:],
                                scalar1=dstrel_sb[:, k:k + 1],
                                scalar2=w_sb[:, k:k + 1],
                                op0=mybir.AluOpType.is_equal,
                                op1=mybir.AluOpType.mult,
                            )
                            nc.tensor.matmul(
                                out=out_ps[:],
                                lhsT=g_t[:, k_rel * P:(k_rel + 1) * P],
                                rhs=mask[:],
                                start=first, stop=False,
                            )
                            first = False
                    # self term
                    if layer == 0:
                        st = xT_sb[:, t * P:(t + 1) * P]
                    else:
                        st_t = sb2.tile([HID, P], F16, tag="st")
                        nc.sync.dma_start(st_t[:], hts[layer - 1][:, t * P:(t + 1) * P])
                        st = st_t[:]
                    nc.tensor.matmul(
                        out=out_ps[:], lhsT=Wr_sb[:], rhs=st,
                        start=False, stop=True,
                    )

                    if layer < 2:
                        hT_sb = sb2.tile([HID, P], F16, tag="hT")
                        nc.scalar.activation(
                            hT_sb[:], out_ps[:],
                            mybir.ActivationFunctionType.Relu, bias=bl_sb[:, :1],
                        )
                        nc.sync.dma_start(hts[layer][:, t * P:(t + 1) * P], hT_sb[:])
                        m_ps = ps.tile([P, HID], F32, tag="mps")
                        nc.tensor.matmul(
                            out=m_ps[:], lhsT=hT_sb[:], rhs=Wl_next[:],
                            start=True, stop=True,
                        )
                        m_sb = sb2.tile([P, HID], F16, tag="msb")
                        nc.vector.tensor_copy(m_sb[:], m_ps[:])
                        nc.sync.dma_start(slabs[layer + 1][t * P:(t + 1) * P, :], m_sb[:])
                    else:
                        h3_sb = sb2.tile([HID, P], F16, tag="hT")
                        nc.vector.tensor_scalar(
                            out=h3_sb[:], in0=out_ps[:],
                            scalar1=bl_sb[:, :1], scalar2=None,
                            op0=mybir.AluOpType.add,
                        )
                        h3rm_ps = ps.tile([P, HID], F16, tag="h3rm")
                        nc.tensor.transpose(h3rm_ps[:], h3_sb[:], ident16[:])
                        h3rm_sb = sb2.tile([P, HID], F16, tag="h3rmsb")
                        nc.vector.tensor_copy(h3rm_sb[:], h3rm_ps[:])
                        gmask = sb2.tile([P, GPC], F16, tag="gmask")
                        nc.vector.tensor_scalar(
                            out=gmask[:], in0=iota_f[:, :GPC],
                            scalar1=br_sb[:, t:t + 1],
                            scalar2=gw_sb[:, t:t + 1],
                            op0=mybir.AluOpType.is_equal,
                            op1=mybir.AluOpType.mult,
                        )
                        nc.tensor.matmul(
                            out=poolT_ps[:], lhsT=h3rm_sb[:], rhs=gmask[:],
                            start=(t == 0), stop=(t == NT - 1),
                        )
                    if "nobarrier" not in ABLATE:
                        tc.no_sync_barrier()

            if layer < 2 and "noag" not in ABLATE:
                nc.gpsimd.collective_compute(
                    "AllGather", mybir.AluOpType.bypass, replica_groups=rg,
                    ins=[slabs[layer + 1].opt()], outs=[tbls[layer + 1].opt()],
                )
                tc.no_sync_barrier()

        # ---- head --------------------------------------------------------
        poolT_sb = sb.tile([HID, GPC], F16)
        nc.vector.tensor_copy(poolT_sb[:], poolT_ps[:])
        fin_ps = pool_ps.tile([cfg.num_classes, GPC], F32, tag="pool")
        nc.tensor.matmul(
            out=fin_ps[:], lhsT=Wlin_sb[:], rhs=poolT_sb[:], start=True, stop=True,
        )
        fin_sb = sb.tile([cfg.num_classes, GPC], F32)
        nc.vector.tensor_scalar(
            out=fin_sb[:], in0=fin_ps[:],
            scalar1=blin_sb[:, :1], scalar2=None,
            op0=mybir.AluOpType.add,
        )
        nc.sync.dma_start(out_d[:, :], fin_sb[:])

    nc.compile()
    return nc


# --------------------------------------------------------------------------
# Host-side preprocessing
# --------------------------------------------------------------------------

def preprocess(x, edge_index, batch, cfg_overrides=None):
    num_nodes = x.shape[0]
    in_feat = x.shape[1]
    num_edges = edge_index.shape[1]
    batch = np.asarray(batch, dtype=np.int64)
    src_all = np.asarray(edge_index[0], dtype=np.int64)
    dst_all = np.asarray(edge_index[1], dtype=np.int64)
    n_cores = 8
    nb = 4
    num_graphs = int(cfg_overrides.get("num_graphs")) if cfg_overrides and "num_graphs" in cfg_overrides else 512
    gpc = num_graphs // n_cores

    # node ranges per core (batch sorted)
    bounds = np.searchsorted(batch, np.arange(n_cores + 1) * gpc)
    nl = bounds[1:] - bounds[:-1]
    blk = nb * P
    n_own = int(-(-int(nl.max()) // blk) * blk)
    assert 2 * n_own <= 32767, "int16 chunk limit"
    chrows = 2 * n_own

    # degrees and edge weights
    deg = np.bincount(dst_all, minlength=num_nodes)
    w_all = np.zeros(num_edges, np.float32)
    nz = deg[dst_all] > 0
    w_all[nz] = 1.0 / deg[dst_all[nz]]

    owner_d = (batch[dst_all] // gpc).astype(np.int64)
    owner_s = (batch[src_all] // gpc).astype(np.int64)
    src_row = (owner_s * n_own + (src_all - bounds[owner_s])).astype(np.int64)
    chunk = src_row // chrows
    src_rel = (src_row - chunk * chrows).astype(np.int16)
    ld = (dst_all - bounds[owner_d]).astype(np.int64)   # local dst
    tile_of = ld // P

    nt = n_own // P
    # group key: (core, tile, chunk)
    gkey = (owner_d * nt + tile_of) * CH + chunk
    ngroups = n_cores * nt * CH
    gcounts = np.bincount(gkey, minlength=ngroups)
    # per-chunk budgets: max over (core, tile)
    cnt3 = gcounts.reshape(n_cores * nt, CH)
    te_c = tuple(int(-(-int(cnt3[:, c].max()) // P)) for c in range(CH))
    te = sum(te_c)
    offc = [sum(te_c[:c]) for c in range(CH)]

    # slot assignment
    order = np.argsort(gkey, kind="stable")
    gk_sorted = gkey[order]
    group_start = np.zeros(ngroups, np.int64)
    group_start[1:] = np.cumsum(gcounts)[:-1]
    rank = np.arange(num_edges) - group_start[gk_sorted]
    t_s = (gk_sorted // CH) % nt
    c_s = gk_sorted % CH
    core_s = gk_sorted // (nt * CH)
    b_s = t_s // nb
    ti_s = t_s % nb
    col = b_s * (nb * te) + np.array(offc)[c_s] * nb + ti_s * np.array(te_c)[c_s] + rank // P
    slot = col * P + rank % P

    nte = nt * te
    e_proc = nte * P
    idx_arr = np.zeros((n_cores, e_proc), np.int16)
    dstrel_arr = np.full((n_cores, e_proc), DST_SENTINEL, np.float32)
    w_arr = np.zeros((n_cores, e_proc), np.float32)

    eo = order
    idx_arr[core_s, slot] = src_rel[eo]
    dstrel_arr[core_s, slot] = (ld[eo] - tile_of[eo] * P).astype(np.float32)
    w_arr[core_s, slot] = w_all[eo]

    def to_pb(a):
        # [e_proc] -> [128, NTE]: slot s -> [s%128, s//128]
        return np.ascontiguousarray(a.reshape(nte, P).T)

    def to_i16(a):
        # [e_proc] -> [128, NTE*8]: slot s -> [s%16 (+16r), s//16]
        band = a.reshape(e_proc // 16, 16).T
        return np.ascontiguousarray(np.tile(band, (8, 1)))

    per_core = []
    gsizes = np.bincount(batch, minlength=num_graphs).astype(np.float32)
    for c in range(n_cores):
        n0, n1 = int(bounds[c]), int(bounds[c + 1])
        xT = np.zeros((in_feat, n_own), np.float16)
        xT[:, : n1 - n0] = x[n0:n1].T.astype(np.float16)
        br = np.zeros(n_own, np.float32)
        gw = np.zeros(n_own, np.float32)
        br[: n1 - n0] = (batch[n0:n1] - c * gpc).astype(np.float32)
        gs = gsizes[batch[n0:n1]]
        gwv = np.zeros(n1 - n0, np.float32)
        gwv[gs > 0] = 1.0 / gs[gs > 0]
        gw[: n1 - n0] = gwv

        per_core.append(dict(
            xT=xT,
            eidx=to_i16(idx_arr[c]),
            edstrel=to_pb(dstrel_arr[c]),
            ew=to_pb(w_arr[c]),
            brel=np.ascontiguousarray(br.reshape(nt, P).T),
            gw=np.ascontiguousarray(gw.reshape(nt, P).T),
        ))

    cfg = Cfg(
        n_cores=n_cores, num_nodes=num_nodes, num_edges=num_edges,
        in_feat=in_feat, hidden=128, num_graphs=num_graphs,
        num_classes=2, n_own=n_own, te_c=te_c, nb=nb, gpc=gpc,
    )
    return cfg, per_core


def make_in_maps(cfg, per_core, weights):
    wmap = {}
    for k in ("Wl1", "Wr1", "Wl2", "Wr2", "Wl3", "Wr3", "Wlin"):
        wmap[k] = np.ascontiguousarray(weights[k].astype(np.float16))
    for k in ("bl1", "bl2", "bl3", "blin"):
        wmap[k] = np.ascontiguousarray(weights[k].astype(np.float32).reshape(-1, 1))
    in_maps = []
    for c in range(cfg.n_cores):
        m = dict(per_core[c])
        m.update(wmap)
        in_maps.append(m)
    return in_maps


_PROGRAM_CACHE = {}


def kernel(x, edge_index, batch,
           Wl1, bl1, Wr1, Wl2, bl2, Wr2, Wl3, bl3, Wr3, Wlin, blin):
    x = np.asarray(x)
    cfg, per_core = preprocess(np.asarray(x, np.float32),
                               np.asarray(edge_index), np.asarray(batch))
    weights = dict(Wl1=np.asarray(Wl1), bl1=np.asarray(bl1), Wr1=np.asarray(Wr1),
                   Wl2=np.asarray(Wl2), bl2=np.asarray(bl2), Wr2=np.asarray(Wr2),
                   Wl3=np.asarray(Wl3), bl3=np.asarray(bl3), Wr3=np.asarray(Wr3),
                   Wlin=np.asarray(Wlin), blin=np.asarray(blin))
    in_maps = make_in_maps(cfg, per_core, weights)

    key = (cfg.n_own, cfg.te_c, cfg.in_feat, cfg.num_graphs)
    if key not in _PROGRAM_CACHE:
        _PROGRAM_CACHE[key] = build_program(cfg)
    nc = _PROGRAM_CACHE[key]

    res = bass_utils.run_bass_kernel_spmd(
        nc, in_maps, core_ids=list(range(cfg.n_cores)),
    )
    out = np.empty((cfg.num_graphs, cfg.num_classes), np.float32)
    for c in range(cfg.n_cores):
        out[c * cfg.gpc:(c + 1) * cfg.gpc, :] = res.results[c]["out"].T
    return out



# revision 2
# speedup vs baseline: 1.1252x; 1.1252x over previous
"""Distributed GraphSAGE kernel for Trainium2 (8 NeuronCores, Bass/Tile) — v2.

Row-major aggregation scheme:
  - table row v (512B fp16): [ h_v @ Wl_next (128) | maxdeg_v * (h_v @ Wr_next) (128) ]
  - gather (SWDGE dma_gather, elem 256B, elem_step 512B) fetches only the agg
    half of m[src] for each edge slot; exact per-(tile,chunk) subtile counts
  - per dst tile: PSUM[dst,f] += onehot_mask(lhsT) @ g(rhs) over subtiles,
    += ident @ self_rows (slab-local), += maxdeg-row (x) bias-row (rank-1)
  - ACT: h = relu(PSUM * (1/maxdeg))  (per-partition scale = per-dst-node)
  - table build: hT via PE transpose, M = hT^T @ [Wl|Wr], self half scaled by
    maxdeg during the PSUM->SBUF fp16 copy, slab written row-major, AllGather
    (Shared output) -> next table
  - layer 3: h3 = PSUM * invdeg (identity act), pooling via host-shipped
    per-tile graph one-hot/gsize masks: poolT[f,G] += h3(lhsT) @ gmask(rhs)
"""
import sys

sys.path.insert(0, "/opt/trn_rl_repo")

import numpy as np
from contextlib import ExitStack
from dataclasses import dataclass

from concourse import bass, mybir, tile, bacc
from concourse import bass_utils
from concourse.masks import make_identity

P = 128
CH = 4              # table row chunks (int16 index limit)
F16 = mybir.dt.float16
F32 = mybir.dt.float32
I16 = mybir.dt.int16

DST_SENTINEL = -300.0
MASK_ENGINE = "split"   # 'dve' | 'act' | 'split'


@dataclass(frozen=True)
class Cfg:
    n_cores: int
    num_nodes: int
    num_edges: int
    in_feat: int
    hidden: int
    num_graphs: int
    num_classes: int
    n_own: int              # padded nodes per core (multiple of NB*128)
    nb: int                 # node tiles per gather batch
    gpc: int                # graphs per core
    k_tc: tuple             # per (tile, chunk) subtile counts, len NT*CH

    @property
    def nt(self):
        return self.n_own // P

    @property
    def nsub(self):
        return sum(self.k_tc)


def build_program(cfg: Cfg):
    nc = bacc.Bacc(
        "TRN2",
        target_bir_lowering=False,
        debug=False,
        num_devices=cfg.n_cores,
        num_swdge_queues=1,
    )

    NT, NB = cfg.nt, cfg.nb
    NSUB = cfg.nsub
    HID = cfg.hidden
    H2 = 2 * HID
    INF = cfg.in_feat
    GPC = cfg.gpc
    NC = cfg.n_cores
    CHROWS = 2 * cfg.n_own
    NBATCH = NT // NB
    KTC = cfg.k_tc          # [t*CH + c]

    # derived layout: subtile order (b, c, t, j)
    # per (b,c): k_bc = sum over tiles in batch; call idx cols contiguous
    sub_of = {}             # (t, c, j) -> global subtile index
    s = 0
    call_meta = []          # per (b, c): (s0, k_bc)
    batch_width = []        # per b: total subtiles
    for b in range(NBATCH):
        w = 0
        for c in range(CH):
            s0 = s
            for ti in range(NB):
                t = b * NB + ti
                for j in range(KTC[t * CH + c]):
                    sub_of[(t, c, j)] = s
                    s += 1
            call_meta.append((s0, s - s0))
            w += s - s0
        batch_width.append(w)
    assert s == NSUB
    GMAX = max(batch_width) if batch_width else 1

    # ---- I/O -------------------------------------------------------------
    xT_d = nc.dram_tensor("xT", [INF, cfg.n_own], F16, kind="ExternalInput")
    idx_d = nc.dram_tensor("eidx", [P, NSUB * 8], I16, kind="ExternalInput")
    dstrel_d = nc.dram_tensor("edstrel", [P, NSUB], F32, kind="ExternalInput")
    ndstrel_d = nc.dram_tensor("endstrel", [P, NSUB], F32, kind="ExternalInput")
    invdeg_d = nc.dram_tensor("invdeg", [P, NT], F32, kind="ExternalInput")
    degrow_d = nc.dram_tensor("degrow", [1, cfg.n_own], F16, kind="ExternalInput")
    gmask_d = nc.dram_tensor("gmask", [P, NT * GPC], F16, kind="ExternalInput")
    degcol_d = nc.dram_tensor("degcol", [P, NT], F32, kind="ExternalInput")
    WlWr1_d = nc.dram_tensor("WlWr1", [INF, H2], F16, kind="ExternalInput")
    WlWr2_d = nc.dram_tensor("WlWr2", [HID, H2], F16, kind="ExternalInput")
    WlWr3_d = nc.dram_tensor("WlWr3", [HID, H2], F16, kind="ExternalInput")
    Wlin_d = nc.dram_tensor("Wlin", [HID, cfg.num_classes], F16, kind="ExternalInput")
    brow1_d = nc.dram_tensor("brow1", [1, HID], F16, kind="ExternalInput")
    brow2_d = nc.dram_tensor("brow2", [1, HID], F16, kind="ExternalInput")
    brow3_d = nc.dram_tensor("brow3", [1, HID], F16, kind="ExternalInput")
    blinrow_d = nc.dram_tensor("blinrow", [1, cfg.num_classes], F16, kind="ExternalInput")
    onesrow_d = nc.dram_tensor("onesrow", [1, GPC], F16, kind="ExternalInput")
    out_d = nc.dram_tensor("out", [cfg.num_classes, GPC], F32, kind="ExternalOutput")

    rg = [list(range(NC))]

    with tile.TileContext(nc) as tc, ExitStack() as ctx:
        sb = ctx.enter_context(tc.tile_pool(name="sb", bufs=1))
        mk = ctx.enter_context(tc.tile_pool(name="mk", bufs=8))
        hb = ctx.enter_context(tc.tile_pool(name="hb", bufs=4))
        gbuf = ctx.enter_context(tc.tile_pool(name="gbuf", bufs=2))
        ps = ctx.enter_context(tc.tile_pool(name="ps", bufs=2, space="PSUM"))
        ps2 = ctx.enter_context(tc.tile_pool(name="ps2", bufs=2, space="PSUM"))
        pool_ps = ctx.enter_context(tc.tile_pool(name="pps", bufs=1, space="PSUM"))
        dram = ctx.enter_context(tc.tile_pool(name="dram", bufs=1, space="DRAM"))

        # ---- static SBUF state ------------------------------------------
        iota_i = sb.tile([P, P], mybir.dt.int32)
        nc.gpsimd.iota(iota_i[:], pattern=[[1, P]], base=0, channel_multiplier=0)
        iota_f = sb.tile([P, P], F32)
        nc.vector.tensor_copy(iota_f[:], iota_i[:])
        iota_h = sb.tile([P, P], F16)
        nc.vector.tensor_copy(iota_h[:], iota_i[:])
        ident16 = sb.tile([P, P], F16)
        make_identity(nc, ident16[:])

        idx_sb = sb.tile([P, NSUB * 8], I16)
        nc.sync.dma_start(idx_sb[:], idx_d[:, :])
        dstrel_sb = sb.tile([P, NSUB], F32)
        nc.sync.dma_start(dstrel_sb[:], dstrel_d[:, :])
        ndstrel_sb = sb.tile([P, NSUB], F32)
        nc.sync.dma_start(ndstrel_sb[:], ndstrel_d[:, :])
        invdeg_sb = sb.tile([P, NT], F32)
        nc.sync.dma_start(invdeg_sb[:], invdeg_d[:, :])
        degrow_sb = sb.tile([1, cfg.n_own], F16)
        nc.sync.dma_start(degrow_sb[:], degrow_d[:, :])
        gmask_sb = sb.tile([P, NT * GPC], F16)
        nc.sync.dma_start(gmask_sb[:], gmask_d[:, :])
        degcol_sb = sb.tile([P, NT], F32)
        nc.sync.dma_start(degcol_sb[:], degcol_d[:, :])

        def load_w(d, p_, f_, nm):
            t = sb.tile([p_, f_], F16, name=nm, tag=nm)
            nc.sync.dma_start(t[:], d[:, :])
            return t

        WlWr1_sb = load_w(WlWr1_d, INF, H2, "w1s")
        WlWr2_sb = load_w(WlWr2_d, HID, H2, "w2s")
        WlWr3_sb = load_w(WlWr3_d, HID, H2, "w3s")
        Wlin_sb = load_w(Wlin_d, HID, cfg.num_classes, "wlins")
        brow1_sb = load_w(brow1_d, 1, HID, "b1s")
        brow2_sb = load_w(brow2_d, 1, HID, "b2s")
        brow3_sb = load_w(brow3_d, 1, HID, "b3s")
        blinrow_sb = load_w(blinrow_d, 1, cfg.num_classes, "bls")
        onesrow_sb = load_w(onesrow_d, 1, GPC, "o1s")

        xT_sb = sb.tile([INF, cfg.n_own], F16)
        nc.sync.dma_start(xT_sb[:], xT_d[:, :])
        tc.no_sync_barrier()

        # ---- internal DRAM ----------------------------------------------
        slab = dram.tile([cfg.n_own, H2], F16, tag="slab", name="slab")
        tbls = [dram.tile([NC * cfg.n_own, H2], F16, tag=f"tbl{l}",
                          name=f"tbl{l}", addr_space="Shared")
                for l in range(3)]

        def build_table_tile(t, lhsT, W_sb, first_layer):
            """M = lhs @ [Wl|Wr]; self half scaled by maxdeg; write slab rows."""
            m_ps = ps2.tile([P, H2], F32, tag="mps")
            nc.tensor.matmul(out=m_ps[:], lhsT=lhsT, rhs=W_sb[:],
                             start=True, stop=True)
            m_sb = hb.tile([P, H2], F16, tag="msb")
            # agg half: plain copy; self half: * maxdeg (per free col? no:
            # maxdeg is per NODE = per PARTITION here since m_ps is [node, :])
            nc.scalar.copy(m_sb[:, 0:HID], m_ps[:, 0:HID])
            nc.scalar.mul(m_sb[:, HID:H2], m_ps[:, HID:H2],
                          degcol_sb[:, t:t + 1])
            nc.sync.dma_start(slab[t * P:(t + 1) * P, :], m_sb[:])

        # ---- P0: table1 = x @ [Wl1|Wr1] ---------------------------------
        for t in range(NT):
            build_table_tile(t, xT_sb[:, t * P:(t + 1) * P], WlWr1_sb, True)

        nc.gpsimd.collective_compute(
            "AllGather", mybir.AluOpType.bypass, replica_groups=rg,
            ins=[slab.opt()], outs=[tbls[0].opt()],
        )
        tc.no_sync_barrier()

        # ---- layers ------------------------------------------------------
        for layer in range(3):
            tbl = tbls[layer]
            brow_sb = (brow1_sb, brow2_sb, brow3_sb)[layer]
            W_next = (WlWr2_sb, WlWr3_sb, None)[layer]

            if layer == 2:
                poolT_ps = pool_ps.tile([HID, GPC], F32, tag="pool")

            for b in range(NBATCH):
                g_t = gbuf.tile([P, GMAX * P], F16, tag="g")
                off_bc = []
                off = 0
                for c in range(CH):
                    s0, k_bc = call_meta[b * CH + c]
                    off_bc.append(off)
                    if k_bc == 0:
                        continue
                    nidx = k_bc * P
                    nc.gpsimd.dma_gather(
                        out_ap=g_t[:, off * P:(off + k_bc) * P].rearrange(
                            "p (t e) -> p t e", e=HID),
                        in_ap=tbl[c * CHROWS:(c + 1) * CHROWS, 0:HID],
                        idxs_ap=idx_sb[:, s0 * 8:(s0 + k_bc) * 8],
                        num_idxs=nidx,
                        num_idxs_reg=nidx,
                        elem_size=HID,
                        elem_step=H2,
                        single_packet=False,
                    )
                    off += k_bc

                for ti in range(NB):
                    t = b * NB + ti
                    out_ps = ps.tile([P, HID], F32, tag="agg")
                    first = True
                    si = 0  # DVE/ACT alternation counter
                    for c in range(CH):
                        base = off_bc[c]
                        # tiles before ti in this (b, c) group
                        pre = sum(KTC[(b * NB + u) * CH + c] for u in range(ti))
                        for j in range(KTC[t * CH + c]):
                            scol = sub_of[(t, c, j)]
                            gcol = base + pre + j
                            mask = mk.tile([P, P], F16, tag="mask")
                            use_act = (MASK_ENGINE == "act" or
                                       (MASK_ENGINE == "split" and si % 2 == 1))
                            if use_act:
                                t1 = mk.tile([P, P], F16, tag="mt")
                                nc.scalar.activation(
                                    t1[:], iota_h[:],
                                    mybir.ActivationFunctionType.Abs,
                                    bias=ndstrel_sb[:, scol:scol + 1], scale=1.0,
                                )
                                nc.scalar.activation(
                                    mask[:], t1[:],
                                    mybir.ActivationFunctionType.Relu,
                                    bias=1.0, scale=-1.0,
                                )
                            else:
                                nc.vector.tensor_scalar(
                                    out=mask[:], in0=iota_f[:],
                                    scalar1=dstrel_sb[:, scol:scol + 1],
                                    scalar2=None,
                                    op0=mybir.AluOpType.is_equal,
                                )
                            si += 1
                            nc.tensor.matmul(
                                out=out_ps[:],
                                lhsT=mask[:],
                                rhs=g_t[:, gcol * P:(gcol + 1) * P],
                                start=first, stop=False,
                            )
                            first = False
                    # self rows: slab[t, HID:H2] = maxdeg*(h@Wr)
                    st = hb.tile([P, HID], F16, tag="st")
                    nc.sync.dma_start(st[:], slab[t * P:(t + 1) * P, HID:H2])
                    nc.tensor.matmul(
                        out=out_ps[:], lhsT=ident16[:], rhs=st[:],
                        start=first, stop=False,
                    )
                    # bias: += maxdeg[n] * b[f] (rank-1)
                    nc.tensor.matmul(
                        out=out_ps[:],
                        lhsT=degrow_sb[:, t * P:(t + 1) * P],
                        rhs=brow_sb[:],
                        start=False, stop=True,
                    )

                    h_sb = hb.tile([P, HID], F16, tag="h")
                    nc.scalar.activation(
                        h_sb[:], out_ps[:],
                        (mybir.ActivationFunctionType.Relu if layer < 2
                         else mybir.ActivationFunctionType.Identity),
                        bias=0.0,
                        scale=invdeg_sb[:, t:t + 1],
                    )

                    if layer < 2:
                        # hT via PE transpose, then table build
                        hT_ps = ps2.tile([P, HID], F16, tag="htps")
                        nc.tensor.transpose(hT_ps[:], h_sb[:], ident16[:])
                        hT_sb = hb.tile([P, HID], F16, tag="htsb")
                        nc.scalar.copy(hT_sb[:], hT_ps[:])
                        build_table_tile(t, hT_sb[:], W_next, False)
                    else:
                        nc.tensor.matmul(
                            out=poolT_ps[:], lhsT=h_sb[:],
                            rhs=gmask_sb[:, t * GPC:(t + 1) * GPC],
                            start=(t == 0), stop=(t == NT - 1),
                        )
                tc.no_sync_barrier()

            if layer < 2:
                nc.gpsimd.collective_compute(
                    "AllGather", mybir.AluOpType.bypass, replica_groups=rg,
                    ins=[slab.opt()], outs=[tbls[layer + 1].opt()],
                )
                tc.no_sync_barrier()

        # ---- head --------------------------------------------------------
        poolT_sb = sb.tile([HID, GPC], F16)
        nc.vector.tensor_copy(poolT_sb[:], poolT_ps[:])
        fin_ps = pool_ps.tile([cfg.num_classes, GPC], F32, tag="fin")
        nc.tensor.matmul(
            out=fin_ps[:], lhsT=Wlin_sb[:], rhs=poolT_sb[:],
            start=True, stop=False,
        )
        nc.tensor.matmul(
            out=fin_ps[:], lhsT=blinrow_sb[:], rhs=onesrow_sb[:],
            start=False, stop=True,
        )
        fin_sb = sb.tile([cfg.num_classes, GPC], F32)
        nc.vector.tensor_copy(fin_sb[:], fin_ps[:])
        nc.sync.dma_start(out_d[:, :], fin_sb[:])

    nc.compile()
    return nc


# revision 3
# speedup vs baseline: 1.2838x; 1.1410x over previous
"""Distributed GraphSAGE kernel for Trainium2 (8 NeuronCores, Bass/Tile) — v2.

Row-major aggregation scheme:
  - table row v (512B fp16): [ h_v @ Wl_next (128) | maxdeg_v * (h_v @ Wr_next) (128) ]
  - gather (SWDGE dma_gather, elem 256B, elem_step 512B) fetches only the agg
    half of m[src] for each edge slot; exact per-(tile,chunk) subtile counts
  - per dst tile: PSUM[dst,f] += onehot_mask(lhsT) @ g(rhs) over subtiles,
    += ident @ self_rows (slab-local), += maxdeg-row (x) bias-row (rank-1)
  - ACT: h = relu(PSUM * (1/maxdeg))  (per-partition scale = per-dst-node)
  - table build: hT via PE transpose, M = hT^T @ [Wl|Wr], self half scaled by
    maxdeg during the PSUM->SBUF fp16 copy, slab written row-major, AllGather
    (Shared output) -> next table
  - layer 3: h3 = PSUM * invdeg (identity act), pooling via host-shipped
    per-tile graph one-hot/gsize masks: poolT[f,G] += h3(lhsT) @ gmask(rhs)
"""
import sys

sys.path.insert(0, "/opt/trn_rl_repo")

import numpy as np
from contextlib import ExitStack
from dataclasses import dataclass

from concourse import bass, mybir, tile, bacc
from concourse import bass_utils
from concourse.masks import make_identity

P = 128
CHSPAN = 32768      # table rows per gather chunk (int16 index limit)
F16 = mybir.dt.float16
F32 = mybir.dt.float32
I16 = mybir.dt.int16

DST_SENTINEL = -300.0
MASK_ENGINE = "split"   # 'dve' | 'act' | 'split'


@dataclass(frozen=True)
class Cfg:
    n_cores: int
    num_nodes: int
    num_edges: int
    in_feat: int
    hidden: int
    num_graphs: int
    num_classes: int
    n_own: int              # padded nodes per core (multiple of NB*128)
    nb: int                 # node tiles per gather batch
    gpc: int                # graphs per core
    k_tc: tuple             # per (tile, chunk) subtile counts, len NT*CH

    @property
    def nt(self):
        return self.n_own // P

    @property
    def ch(self):
        return -(-(self.n_cores * self.n_own) // CHSPAN)

    @property
    def nsub(self):
        return sum(self.k_tc)


def build_program(cfg: Cfg):
    nc = bacc.Bacc(
        "TRN2",
        target_bir_lowering=False,
        debug=False,
        num_devices=cfg.n_cores,
        num_swdge_queues=1,
    )

    NT, NB = cfg.nt, cfg.nb
    NSUB = cfg.nsub
    HID = cfg.hidden
    H2 = 2 * HID
    INF = cfg.in_feat
    GPC = cfg.gpc
    NC = cfg.n_cores
    CH = cfg.ch
    TROWS = NC * cfg.n_own
    NBATCH = NT // NB
    KTC = cfg.k_tc          # [t*CH + c]

    # derived layout: subtile order (b, c, t, j)
    # per (b,c): k_bc = sum over tiles in batch; call idx cols contiguous
    sub_of = {}             # (t, c, j) -> global subtile index
    s = 0
    call_meta = []          # per (b, c): (s0, k_bc)
    batch_width = []        # per b: total subtiles
    for b in range(NBATCH):
        w = 0
        for c in range(CH):
            s0 = s
            for ti in range(NB):
                t = b * NB + ti
                for j in range(KTC[t * CH + c]):
                    sub_of[(t, c, j)] = s
                    s += 1
            call_meta.append((s0, s - s0))
            w += s - s0
        batch_width.append(w)
    assert s == NSUB
    GMAX = max(batch_width) if batch_width else 1

    # ---- I/O -------------------------------------------------------------
    xT_d = nc.dram_tensor("xT", [INF, cfg.n_own], F16, kind="ExternalInput")
    idx_d = nc.dram_tensor("eidx", [P, NSUB * 8], I16, kind="ExternalInput")
    masks_d = nc.dram_tensor("emasks", [P, NSUB * P], F16, kind="ExternalInput")
    invdeg_d = nc.dram_tensor("invdeg", [P, NT], F32, kind="ExternalInput")
    degrow_d = nc.dram_tensor("degrow", [1, cfg.n_own], F16, kind="ExternalInput")
    gmask_d = nc.dram_tensor("gmask", [P, NT * GPC], F16, kind="ExternalInput")
    degcol_d = nc.dram_tensor("degcol", [P, NT], F32, kind="ExternalInput")
    WlWr1_d = nc.dram_tensor("WlWr1", [INF, H2], F16, kind="ExternalInput")
    WlWr2_d = nc.dram_tensor("WlWr2", [HID, H2], F16, kind="ExternalInput")
    WlWr3_d = nc.dram_tensor("WlWr3", [HID, H2], F16, kind="ExternalInput")
    Wlin_d = nc.dram_tensor("Wlin", [HID, cfg.num_classes], F16, kind="ExternalInput")
    brow1_d = nc.dram_tensor("brow1", [1, HID], F16, kind="ExternalInput")
    brow2_d = nc.dram_tensor("brow2", [1, HID], F16, kind="ExternalInput")
    brow3_d = nc.dram_tensor("brow3", [1, HID], F16, kind="ExternalInput")
    blinrow_d = nc.dram_tensor("blinrow", [1, cfg.num_classes], F16, kind="ExternalInput")
    onesrow_d = nc.dram_tensor("onesrow", [1, GPC], F16, kind="ExternalInput")
    out_d = nc.dram_tensor("out", [cfg.num_classes, GPC], F32, kind="ExternalOutput")

    rg = [list(range(NC))]

    with tile.TileContext(nc) as tc, ExitStack() as ctx:
        sb = ctx.enter_context(tc.tile_pool(name="sb", bufs=1))
        mk = ctx.enter_context(tc.tile_pool(name="mk", bufs=2))
        hb = ctx.enter_context(tc.tile_pool(name="hb", bufs=4))
        gbuf = ctx.enter_context(tc.tile_pool(name="gbuf", bufs=2))
        ps = ctx.enter_context(tc.tile_pool(name="ps", bufs=2, space="PSUM"))
        ps2 = ctx.enter_context(tc.tile_pool(name="ps2", bufs=2, space="PSUM"))
        pool_ps = ctx.enter_context(tc.tile_pool(name="pps", bufs=1, space="PSUM"))
        dram = ctx.enter_context(tc.tile_pool(name="dram", bufs=1, space="DRAM"))

        # ---- static SBUF state ------------------------------------------
        ident16 = sb.tile([P, P], F16)
        make_identity(nc, ident16[:])

        idx_sb = sb.tile([P, NSUB * 8], I16)
        nc.sync.dma_start(idx_sb[:], idx_d[:, :])
        invdeg_sb = sb.tile([P, NT], F32)
        nc.sync.dma_start(invdeg_sb[:], invdeg_d[:, :])
        degrow_sb = sb.tile([1, cfg.n_own], F16)
        nc.sync.dma_start(degrow_sb[:], degrow_d[:, :])
        gmask_sb = sb.tile([P, NT * GPC], F16)
        nc.sync.dma_start(gmask_sb[:], gmask_d[:, :])
        degcol_sb = sb.tile([P, NT], F32)
        nc.sync.dma_start(degcol_sb[:], degcol_d[:, :])

        def load_w(d, p_, f_, nm):
            t = sb.tile([p_, f_], F16, name=nm, tag=nm)
            nc.sync.dma_start(t[:], d[:, :])
            return t

        WlWr1_sb = load_w(WlWr1_d, INF, H2, "w1s")
        WlWr2_sb = load_w(WlWr2_d, HID, H2, "w2s")
        WlWr3_sb = load_w(WlWr3_d, HID, H2, "w3s")
        Wlin_sb = load_w(Wlin_d, HID, cfg.num_classes, "wlins")
        brow1_sb = load_w(brow1_d, 1, HID, "b1s")
        brow2_sb = load_w(brow2_d, 1, HID, "b2s")
        brow3_sb = load_w(brow3_d, 1, HID, "b3s")
        blinrow_sb = load_w(blinrow_d, 1, cfg.num_classes, "bls")
        onesrow_sb = load_w(onesrow_d, 1, GPC, "o1s")

        xT_sb = sb.tile([INF, cfg.n_own], F16)
        nc.sync.dma_start(xT_sb[:], xT_d[:, :])
        for _ in range(2):
            g0 = gbuf.tile([P, GMAX * P], F16, tag="g")
            nc.vector.memset(g0[:], 0.0)
        tc.no_sync_barrier()

        # ---- internal DRAM ----------------------------------------------
        slab_agg = dram.tile([cfg.n_own, HID], F16, tag="slaba", name="slaba")
        slab_self = dram.tile([cfg.n_own, HID], F16, tag="slabs", name="slabs")
        tbls = [dram.tile([NC * cfg.n_own, HID], F16, tag=f"tbl{l}",
                          name=f"tbl{l}", addr_space="Shared")
                for l in range(3)]

        def build_table_tile(t, lhsT, W_sb, first_layer):
            """M = lhs @ [Wl|Wr]; self half scaled by maxdeg; write slab rows."""
            m_ps = ps2.tile([P, H2], F32, tag="mps")
            nc.tensor.matmul(out=m_ps[:], lhsT=lhsT, rhs=W_sb[:],
                             start=True, stop=True)
            m_sb = hb.tile([P, H2], F16, tag="msb")
            # agg half: plain copy; self half: * maxdeg (per free col? no:
            # maxdeg is per NODE = per PARTITION here since m_ps is [node, :])
            nc.scalar.copy(m_sb[:, 0:HID], m_ps[:, 0:HID])
            nc.scalar.mul(m_sb[:, HID:H2], m_ps[:, HID:H2],
                          degcol_sb[:, t:t + 1])
            nc.sync.dma_start(slab_agg[t * P:(t + 1) * P, :], m_sb[:, 0:HID])
            nc.sync.dma_start(slab_self[t * P:(t + 1) * P, :], m_sb[:, HID:H2])

        # ---- P0: table1 = x @ [Wl1|Wr1] ---------------------------------
        for t in range(NT):
            build_table_tile(t, xT_sb[:, t * P:(t + 1) * P], WlWr1_sb, True)

        nc.gpsimd.collective_compute(
            "AllGather", mybir.AluOpType.bypass, replica_groups=rg,
            ins=[slab_agg.opt()], outs=[tbls[0].opt()],
        )
        tc.no_sync_barrier()

        # ---- layers ------------------------------------------------------
        for layer in range(3):
            tbl = tbls[layer]
            brow_sb = (brow1_sb, brow2_sb, brow3_sb)[layer]
            W_next = (WlWr2_sb, WlWr3_sb, None)[layer]

            if layer == 2:
                poolT_ps = pool_ps.tile([HID, GPC], F32, tag="pool")

            for b in range(NBATCH):
                g_t = gbuf.tile([P, GMAX * P], F16, tag="g")
                mk_t = mk.tile([P, GMAX * P], F16, tag="mk")
                bs0 = call_meta[b * CH][0]       # first subtile of batch
                bw = batch_width[b]
                if bw:
                    nc.sync.dma_start(
                        mk_t[:, :bw * P],
                        masks_d[:, bs0 * P:(bs0 + bw) * P])
                off_bc = []
                off = 0
                for c in range(CH):
                    s0, k_bc = call_meta[b * CH + c]
                    off_bc.append(off)
                    if k_bc == 0:
                        continue
                    nidx = k_bc * P
                    nc.gpsimd.dma_gather(
                        out_ap=g_t[:, off * P:(off + k_bc) * P].rearrange(
                            "p (t e) -> p t e", e=HID),
                        in_ap=tbl[c * CHSPAN:min((c + 1) * CHSPAN, TROWS), :],
                        idxs_ap=idx_sb[:, s0 * 8:(s0 + k_bc) * 8],
                        num_idxs=nidx,
                        num_idxs_reg=nidx,
                        elem_size=HID,
                        single_packet=False,
                    )
                    off += k_bc

                for ti in range(NB):
                    t = b * NB + ti
                    out_ps = ps.tile([P, HID], F32, tag="agg")
                    first = True
                    for c in range(CH):
                        base = off_bc[c]
                        # tiles before ti in this (b, c) group
                        pre = sum(KTC[(b * NB + u) * CH + c] for u in range(ti))
                        for j in range(KTC[t * CH + c]):
                            scol = sub_of[(t, c, j)]
                            gcol = base + pre + j
                            mcol = scol - bs0
                            nc.tensor.matmul(
                                out=out_ps[:],
                                lhsT=mk_t[:, mcol * P:(mcol + 1) * P],
                                rhs=g_t[:, gcol * P:(gcol + 1) * P],
                                start=first, stop=False,
                            )
                            first = False
                    # self rows: slab[t, HID:H2] = maxdeg*(h@Wr)
                    st = hb.tile([P, HID], F16, tag="st")
                    nc.sync.dma_start(st[:], slab_self[t * P:(t + 1) * P, :])
                    nc.tensor.matmul(
                        out=out_ps[:], lhsT=ident16[:], rhs=st[:],
                        start=first, stop=False,
                    )
                    # bias: += maxdeg[n] * b[f] (rank-1)
                    nc.tensor.matmul(
                        out=out_ps[:],
                        lhsT=degrow_sb[:, t * P:(t + 1) * P],
                        rhs=brow_sb[:],
                        start=False, stop=True,
                    )

                    h_sb = hb.tile([P, HID], F16, tag="h")
                    nc.scalar.activation(
                        h_sb[:], out_ps[:],
                        (mybir.ActivationFunctionType.Relu if layer < 2
                         else mybir.ActivationFunctionType.Identity),
                        bias=0.0,
                        scale=invdeg_sb[:, t:t + 1],
                    )

                    if layer < 2:
                        # hT via PE transpose, then table build
                        hT_ps = ps2.tile([P, HID], F16, tag="htps")
                        nc.tensor.transpose(hT_ps[:], h_sb[:], ident16[:])
                        hT_sb = hb.tile([P, HID], F16, tag="htsb")
                        nc.scalar.copy(hT_sb[:], hT_ps[:])
                        build_table_tile(t, hT_sb[:], W_next, False)
                    else:
                        nc.tensor.matmul(
                            out=poolT_ps[:], lhsT=h_sb[:],
                            rhs=gmask_sb[:, t * GPC:(t + 1) * GPC],
                            start=(t == 0), stop=(t == NT - 1),
                        )
                tc.no_sync_barrier()

            if layer < 2:
                nc.gpsimd.collective_compute(
                    "AllGather", mybir.AluOpType.bypass, replica_groups=rg,
                    ins=[slab_agg.opt()], outs=[tbls[layer + 1].opt()],
                )
                tc.no_sync_barrier()

        # ---- head --------------------------------------------------------
        poolT_sb = sb.tile([HID, GPC], F16)
        nc.vector.tensor_copy(poolT_sb[:], poolT_ps[:])
        fin_ps = pool_ps.tile([cfg.num_classes, GPC], F32, tag="fin")
        nc.tensor.matmul(
            out=fin_ps[:], lhsT=Wlin_sb[:], rhs=poolT_sb[:],
            start=True, stop=False,
        )
        nc.tensor.matmul(
            out=fin_ps[:], lhsT=blinrow_sb[:], rhs=onesrow_sb[:],
            start=False, stop=True,
        )
        fin_sb = sb.tile([cfg.num_classes, GPC], F32)
        nc.vector.tensor_copy(fin_sb[:], fin_ps[:])
        nc.sync.dma_start(out_d[:, :], fin_sb[:])

    nc.compile()
    return nc


# revision 4
# speedup vs baseline: 1.3470x; 1.0492x over previous
"""Distributed GraphSAGE kernel for Trainium2 (8 NeuronCores, Bass/Tile) — v2.

Row-major aggregation scheme:
  - table row v (512B fp16): [ h_v @ Wl_next (128) | maxdeg_v * (h_v @ Wr_next) (128) ]
  - gather (SWDGE dma_gather, elem 256B, elem_step 512B) fetches only the agg
    half of m[src] for each edge slot; exact per-(tile,chunk) subtile counts
  - per dst tile: PSUM[dst,f] += onehot_mask(lhsT) @ g(rhs) over subtiles,
    += ident @ self_rows (slab-local), += maxdeg-row (x) bias-row (rank-1)
  - ACT: h = relu(PSUM * (1/maxdeg))  (per-partition scale = per-dst-node)
  - table build: hT via PE transpose, M = hT^T @ [Wl|Wr], self half scaled by
    maxdeg during the PSUM->SBUF fp16 copy, slab written row-major, AllGather
    (Shared output) -> next table
  - layer 3: h3 = PSUM * invdeg (identity act), pooling via host-shipped
    per-tile graph one-hot/gsize masks: poolT[f,G] += h3(lhsT) @ gmask(rhs)
"""
import sys

sys.path.insert(0, "/opt/trn_rl_repo")

import numpy as np
from contextlib import ExitStack
from dataclasses import dataclass

from concourse import bass, mybir, tile, bacc
from concourse import bass_utils
from concourse.masks import make_identity

P = 128
CHSPAN = 32768      # table rows per gather chunk (int16 index limit)
F16 = mybir.dt.float16
F32 = mybir.dt.float32
I16 = mybir.dt.int16

DST_SENTINEL = -300.0
MASK_ENGINE = "split"   # 'dve' | 'act' | 'split'


@dataclass(frozen=True)
class Cfg:
    n_cores: int
    num_nodes: int
    num_edges: int
    in_feat: int
    hidden: int
    num_graphs: int
    num_classes: int
    n_own: int              # padded nodes per core (multiple of NB*128)
    nb: int                 # node tiles per gather batch
    gpc: int                # graphs per core
    k_tc: tuple             # per (tile, chunk) subtile counts, len NT*CH

    @property
    def nt(self):
        return self.n_own // P

    @property
    def half(self):
        return self.n_own // 2

    @property
    def ch_per_half(self):
        return -(-(self.n_cores * self.half) // CHSPAN)

    @property
    def ch(self):
        return 2 * self.ch_per_half

    @property
    def nsub(self):
        return sum(self.k_tc)


def build_program(cfg: Cfg):
    nc = bacc.Bacc(
        "TRN2",
        target_bir_lowering=False,
        debug=False,
        num_devices=cfg.n_cores,
        num_swdge_queues=1,
    )

    NT, NB = cfg.nt, cfg.nb
    NSUB = cfg.nsub
    HID = cfg.hidden
    H2 = 2 * HID
    INF = cfg.in_feat
    GPC = cfg.gpc
    NC = cfg.n_cores
    CH = cfg.ch
    CPH = cfg.ch_per_half
    HALF = cfg.half
    HROWS = NC * HALF
    NBATCH = NT // NB
    AGA_TILE = ((NT // 2 + NB - 1) // NB) * NB - 1   # last tile of AG_A batch
    AGA_BATCH = AGA_TILE // NB
    KTC = cfg.k_tc          # [t*CH + c]

    # derived layout: subtile order (b, c, t, j)
    # per (b,c): k_bc = sum over tiles in batch; call idx cols contiguous
    sub_of = {}             # (t, c, j) -> global subtile index
    s = 0
    call_meta = []          # per (b, c): (s0, k_bc)
    batch_width = []        # per b: total subtiles
    for b in range(NBATCH):
        w = 0
        for c in range(CH):
            s0 = s
            for ti in range(NB):
                t = b * NB + ti
                for j in range(KTC[t * CH + c]):
                    sub_of[(t, c, j)] = s
                    s += 1
            call_meta.append((s0, s - s0))
            w += s - s0
        batch_width.append(w)
    assert s == NSUB
    GMAX = max(batch_width) if batch_width else 1

    # ---- I/O -------------------------------------------------------------
    xT_d = nc.dram_tensor("xT", [INF, cfg.n_own], F16, kind="ExternalInput")
    idx_d = nc.dram_tensor("eidx", [P, NSUB * 8], I16, kind="ExternalInput")
    masks_d = nc.dram_tensor("emasks", [P, NSUB * P], F16, kind="ExternalInput")
    invdeg_d = nc.dram_tensor("invdeg", [P, NT], F32, kind="ExternalInput")
    degrow_d = nc.dram_tensor("degrow", [1, cfg.n_own], F16, kind="ExternalInput")
    gmask_d = nc.dram_tensor("gmask", [P, NT * GPC], F16, kind="ExternalInput")
    degcol_d = nc.dram_tensor("degcol", [P, NT], F32, kind="ExternalInput")
    WlWr1_d = nc.dram_tensor("WlWr1", [INF, H2], F16, kind="ExternalInput")
    WlWr2_d = nc.dram_tensor("WlWr2", [HID, H2], F16, kind="ExternalInput")
    WlWr3_d = nc.dram_tensor("WlWr3", [HID, H2], F16, kind="ExternalInput")
    Wlin_d = nc.dram_tensor("Wlin", [HID, cfg.num_classes], F16, kind="ExternalInput")
    brow1_d = nc.dram_tensor("brow1", [1, HID], F16, kind="ExternalInput")
    brow2_d = nc.dram_tensor("brow2", [1, HID], F16, kind="ExternalInput")
    brow3_d = nc.dram_tensor("brow3", [1, HID], F16, kind="ExternalInput")
    blinrow_d = nc.dram_tensor("blinrow", [1, cfg.num_classes], F16, kind="ExternalInput")
    onesrow_d = nc.dram_tensor("onesrow", [1, GPC], F16, kind="ExternalInput")
    out_d = nc.dram_tensor("out", [cfg.num_classes, GPC], F32, kind="ExternalOutput")

    rg = [list(range(NC))]

    with tile.TileContext(nc) as tc, ExitStack() as ctx:
        sb = ctx.enter_context(tc.tile_pool(name="sb", bufs=1))
        mk = ctx.enter_context(tc.tile_pool(name="mk", bufs=2))
        hb = ctx.enter_context(tc.tile_pool(name="hb", bufs=4))
        gbuf = ctx.enter_context(tc.tile_pool(name="gbuf", bufs=2))
        ps = ctx.enter_context(tc.tile_pool(name="ps", bufs=2, space="PSUM"))
        ps2 = ctx.enter_context(tc.tile_pool(name="ps2", bufs=2, space="PSUM"))
        pool_ps = ctx.enter_context(tc.tile_pool(name="pps", bufs=1, space="PSUM"))
        dram = ctx.enter_context(tc.tile_pool(name="dram", bufs=1, space="DRAM"))

        # ---- static SBUF state ------------------------------------------
        ident16 = sb.tile([P, P], F16)
        make_identity(nc, ident16[:])

        idx_sb = sb.tile([P, NSUB * 8], I16)
        nc.sync.dma_start(idx_sb[:], idx_d[:, :])
        invdeg_sb = sb.tile([P, NT], F32)
        nc.sync.dma_start(invdeg_sb[:], invdeg_d[:, :])
        degrow_sb = sb.tile([1, cfg.n_own], F16)
        nc.sync.dma_start(degrow_sb[:], degrow_d[:, :])
        gmask_sb = sb.tile([P, NT * GPC], F16)
        nc.sync.dma_start(gmask_sb[:], gmask_d[:, :])
        degcol_sb = sb.tile([P, NT], F32)
        nc.sync.dma_start(degcol_sb[:], degcol_d[:, :])

        def load_w(d, p_, f_, nm):
            t = sb.tile([p_, f_], F16, name=nm, tag=nm)
            nc.sync.dma_start(t[:], d[:, :])
            return t

        WlWr1_sb = load_w(WlWr1_d, INF, H2, "w1s")
        WlWr2_sb = load_w(WlWr2_d, HID, H2, "w2s")
        WlWr3_sb = load_w(WlWr3_d, HID, H2, "w3s")
        Wlin_sb = load_w(Wlin_d, HID, cfg.num_classes, "wlins")
        brow1_sb = load_w(brow1_d, 1, HID, "b1s")
        brow2_sb = load_w(brow2_d, 1, HID, "b2s")
        brow3_sb = load_w(brow3_d, 1, HID, "b3s")
        blinrow_sb = load_w(blinrow_d, 1, cfg.num_classes, "bls")
        onesrow_sb = load_w(onesrow_d, 1, GPC, "o1s")

        xT_sb = sb.tile([INF, cfg.n_own], F16)
        nc.sync.dma_start(xT_sb[:], xT_d[:, :])
        for _ in range(2):
            g0 = gbuf.tile([P, GMAX * P], F16, tag="g")
            nc.vector.memset(g0[:], 0.0)
        tc.no_sync_barrier()

        # ---- internal DRAM ----------------------------------------------
        slab_agg = dram.tile([cfg.n_own, HID], F16, tag="slaba", name="slaba")
        slab_self = dram.tile([cfg.n_own, HID], F16, tag="slabs", name="slabs")
        tbls = [[dram.tile([HROWS, HID], F16, tag=f"tbl{l}{h}",
                           name=f"tbl{l}{h}", addr_space="Shared")
                 for h in range(2)] for l in range(3)]

        def ag_half(lyr, h):
            nc.gpsimd.collective_compute(
                "AllGather", mybir.AluOpType.bypass, replica_groups=rg,
                ins=[slab_agg[h * HALF:(h + 1) * HALF, :]],
                outs=[tbls[lyr][h].opt()],
            )

        def build_table_tile(t, lhsT, W_sb, first_layer):
            """M = lhs @ [Wl|Wr]; self half scaled by maxdeg; write slab rows."""
            m_ps = ps2.tile([P, H2], F32, tag="mps")
            nc.tensor.matmul(out=m_ps[:], lhsT=lhsT, rhs=W_sb[:],
                             start=True, stop=True)
            m_sb = hb.tile([P, H2], F16, tag="msb")
            # agg half: plain copy; self half: * maxdeg (per free col? no:
            # maxdeg is per NODE = per PARTITION here since m_ps is [node, :])
            nc.scalar.copy(m_sb[:, 0:HID], m_ps[:, 0:HID])
            nc.scalar.mul(m_sb[:, HID:H2], m_ps[:, HID:H2],
                          degcol_sb[:, t:t + 1])
            nc.sync.dma_start(slab_agg[t * P:(t + 1) * P, :], m_sb[:, 0:HID])
            nc.sync.dma_start(slab_self[t * P:(t + 1) * P, :], m_sb[:, HID:H2])

        # ---- P0: table1 = x @ [Wl1|Wr1] ---------------------------------
        for t in range(NT):
            build_table_tile(t, xT_sb[:, t * P:(t + 1) * P], WlWr1_sb, True)
            if t == NT // 2 - 1:
                ag_half(0, 0)
        ag_half(0, 1)
        tc.no_sync_barrier()

        # ---- layers ------------------------------------------------------
        for layer in range(3):
            tblh = tbls[layer]
            brow_sb = (brow1_sb, brow2_sb, brow3_sb)[layer]
            W_next = (WlWr2_sb, WlWr3_sb, None)[layer]

            if layer == 2:
                poolT_ps = pool_ps.tile([HID, GPC], F32, tag="pool")

            for b in range(NBATCH):
                g_t = gbuf.tile([P, GMAX * P], F16, tag="g")
                mk_t = mk.tile([P, GMAX * P], F16, tag="mk")
                bs0 = call_meta[b * CH][0]       # first subtile of batch
                bw = batch_width[b]
                if bw:
                    nc.sync.dma_start(
                        mk_t[:, :bw * P],
                        masks_d[:, bs0 * P:(bs0 + bw) * P])
                off_bc = []
                off = 0
                for c in range(CH):
                    s0, k_bc = call_meta[b * CH + c]
                    off_bc.append(off)
                    if k_bc == 0:
                        continue
                    nidx = k_bc * P
                    nc.gpsimd.dma_gather(
                        out_ap=g_t[:, off * P:(off + k_bc) * P].rearrange(
                            "p (t e) -> p t e", e=HID),
                        in_ap=tblh[c // CPH][
                            (c % CPH) * CHSPAN:
                            min((c % CPH + 1) * CHSPAN, HROWS), :],
                        idxs_ap=idx_sb[:, s0 * 8:(s0 + k_bc) * 8],
                        num_idxs=nidx,
                        num_idxs_reg=nidx,
                        elem_size=HID,
                        single_packet=False,
                    )
                    off += k_bc

                for ti in range(NB):
                    t = b * NB + ti
                    out_ps = ps.tile([P, HID], F32, tag="agg")
                    first = True
                    for c in range(CH):
                        base = off_bc[c]
                        # tiles before ti in this (b, c) group
                        pre = sum(KTC[(b * NB + u) * CH + c] for u in range(ti))
                        for j in range(KTC[t * CH + c]):
                            scol = sub_of[(t, c, j)]
                            gcol = base + pre + j
                            mcol = scol - bs0
                            nc.tensor.matmul(
                                out=out_ps[:],
                                lhsT=mk_t[:, mcol * P:(mcol + 1) * P],
                                rhs=g_t[:, gcol * P:(gcol + 1) * P],
                                start=first, stop=False,
                            )
                            first = False
                    # self rows: slab[t, HID:H2] = maxdeg*(h@Wr)
                    st = hb.tile([P, HID], F16, tag="st")
                    nc.sync.dma_start(st[:], slab_self[t * P:(t + 1) * P, :])
                    nc.tensor.matmul(
                        out=out_ps[:], lhsT=ident16[:], rhs=st[:],
                        start=first, stop=False,
                    )
                    # bias: += maxdeg[n] * b[f] (rank-1)
                    nc.tensor.matmul(
                        out=out_ps[:],
                        lhsT=degrow_sb[:, t * P:(t + 1) * P],
                        rhs=brow_sb[:],
                        start=False, stop=True,
                    )

                    h_sb = hb.tile([P, HID], F16, tag="h")
                    nc.scalar.activation(
                        h_sb[:], out_ps[:],
                        (mybir.ActivationFunctionType.Relu if layer < 2
                         else mybir.ActivationFunctionType.Identity),
                        bias=0.0,
                        scale=invdeg_sb[:, t:t + 1],
                    )

                    if layer < 2:
                        # hT via PE transpose, then table build
                        hT_ps = ps2.tile([P, HID], F16, tag="htps")
                        nc.tensor.transpose(hT_ps[:], h_sb[:], ident16[:])
                        hT_sb = hb.tile([P, HID], F16, tag="htsb")
                        nc.scalar.copy(hT_sb[:], hT_ps[:])
                        build_table_tile(t, hT_sb[:], W_next, False)
                    else:
                        nc.tensor.matmul(
                            out=poolT_ps[:], lhsT=h_sb[:],
                            rhs=gmask_sb[:, t * GPC:(t + 1) * GPC],
                            start=(t == 0), stop=(t == NT - 1),
                        )
                if layer < 2 and b == AGA_BATCH:
                    ag_half(layer + 1, 0)
                tc.no_sync_barrier()

            if layer < 2:
                ag_half(layer + 1, 1)
                tc.no_sync_barrier()

        # ---- head --------------------------------------------------------
        poolT_sb = sb.tile([HID, GPC], F16)
        nc.vector.tensor_copy(poolT_sb[:], poolT_ps[:])
        fin_ps = pool_ps.tile([cfg.num_classes, GPC], F32, tag="fin")
        nc.tensor.matmul(
            out=fin_ps[:], lhsT=Wlin_sb[:], rhs=poolT_sb[:],
            start=True, stop=False,
        )
        nc.tensor.matmul(
            out=fin_ps[:], lhsT=blinrow_sb[:], rhs=onesrow_sb[:],
            start=False, stop=True,
        )
        fin_sb = sb.tile([cfg.num_classes, GPC], F32)
        nc.vector.tensor_copy(fin_sb[:], fin_ps[:])
        nc.sync.dma_start(out_d[:, :], fin_sb[:])

    nc.compile()
    return nc


# revision 5
# speedup vs baseline: 1.3654x; 1.0137x over previous
"""Distributed GraphSAGE kernel for Trainium2 (8 NeuronCores, Bass/Tile) — v2.

Row-major aggregation scheme:
  - table row v (512B fp16): [ h_v @ Wl_next (128) | maxdeg_v * (h_v @ Wr_next) (128) ]
  - gather (SWDGE dma_gather, elem 256B, elem_step 512B) fetches only the agg
    half of m[src] for each edge slot; exact per-(tile,chunk) subtile counts
  - per dst tile: PSUM[dst,f] += onehot_mask(lhsT) @ g(rhs) over subtiles,
    += ident @ self_rows (slab-local), += maxdeg-row (x) bias-row (rank-1)
  - ACT: h = relu(PSUM * (1/maxdeg))  (per-partition scale = per-dst-node)
  - table build: hT via PE transpose, M = hT^T @ [Wl|Wr], self half scaled by
    maxdeg during the PSUM->SBUF fp16 copy, slab written row-major, AllGather
    (Shared output) -> next table
  - layer 3: h3 = PSUM * invdeg (identity act), pooling via host-shipped
    per-tile graph one-hot/gsize masks: poolT[f,G] += h3(lhsT) @ gmask(rhs)
"""
import sys

sys.path.insert(0, "/opt/trn_rl_repo")

import numpy as np
from contextlib import ExitStack
from dataclasses import dataclass

from concourse import bass, mybir, tile, bacc
from concourse import bass_utils
from concourse.masks import make_identity

P = 128
CHSPAN = 32768      # table rows per gather chunk (int16 index limit)
F16 = mybir.dt.float16
F32 = mybir.dt.float32
I16 = mybir.dt.int16

DST_SENTINEL = -300.0
MASK_ENGINE = "split"   # 'dve' | 'act' | 'split'


@dataclass(frozen=True)
class Cfg:
    n_cores: int
    num_nodes: int
    num_edges: int
    in_feat: int
    hidden: int
    num_graphs: int
    num_classes: int
    n_own: int              # padded nodes per core (multiple of NB*128)
    nb: int                 # node tiles per gather batch
    gpc: int                # graphs per core
    k_tc: tuple             # per (tile, chunk) subtile counts, len NT*CH

    @property
    def nt(self):
        return self.n_own // P

    @property
    def half(self):
        return self.n_own // 2

    @property
    def ch_per_half(self):
        return -(-(self.n_cores * self.half) // CHSPAN)

    @property
    def ch(self):
        return 2 * self.ch_per_half

    @property
    def nsub(self):
        return sum(self.k_tc)


def build_program(cfg: Cfg):
    nc = bacc.Bacc(
        "TRN2",
        target_bir_lowering=False,
        debug=False,
        num_devices=cfg.n_cores,
        num_swdge_queues=1,
    )

    NT, NB = cfg.nt, cfg.nb
    NSUB = cfg.nsub
    HID = cfg.hidden
    H2 = 2 * HID
    INF = cfg.in_feat
    GPC = cfg.gpc
    NC = cfg.n_cores
    CH = cfg.ch
    CPH = cfg.ch_per_half
    HALF = cfg.half
    HROWS = NC * HALF
    NBATCH = NT // NB
    AGA_TILE = ((NT // 2 + NB - 1) // NB) * NB - 1   # last tile of AG_A batch
    AGA_BATCH = AGA_TILE // NB
    KTC = cfg.k_tc          # [t*CH + c]

    # derived layout: subtile order (b, c, t, j)
    # per (b,c): k_bc = sum over tiles in batch; call idx cols contiguous
    sub_of = {}             # (t, c, j) -> global subtile index
    s = 0
    call_meta = []          # per (b, c): (s0, k_bc)
    batch_width = []        # per b: total subtiles
    for b in range(NBATCH):
        w = 0
        for c in range(CH):
            s0 = s
            for ti in range(NB):
                t = b * NB + ti
                for j in range(KTC[t * CH + c]):
                    sub_of[(t, c, j)] = s
                    s += 1
            call_meta.append((s0, s - s0))
            w += s - s0
        batch_width.append(w)
    assert s == NSUB
    GMAX = max(batch_width) if batch_width else 1

    # ---- I/O -------------------------------------------------------------
    xT_d = nc.dram_tensor("xT", [INF, cfg.n_own], F16, kind="ExternalInput")
    idx_d = nc.dram_tensor("eidx", [P, NSUB * 8], I16, kind="ExternalInput")
    masks_d = nc.dram_tensor("emasks", [P, NSUB * P], F16, kind="ExternalInput")
    invdeg_d = nc.dram_tensor("invdeg", [P, NT], F32, kind="ExternalInput")
    degrow_d = nc.dram_tensor("degrow", [1, cfg.n_own], F16, kind="ExternalInput")
    gmask_d = nc.dram_tensor("gmask", [P, NT * GPC], F16, kind="ExternalInput")
    degcol_d = nc.dram_tensor("degcol", [P, NT], F32, kind="ExternalInput")
    WlWr1_d = nc.dram_tensor("WlWr1", [INF, H2], F16, kind="ExternalInput")
    WlWr2_d = nc.dram_tensor("WlWr2", [HID, H2], F16, kind="ExternalInput")
    WlWr3_d = nc.dram_tensor("WlWr3", [HID, H2], F16, kind="ExternalInput")
    Wlin_d = nc.dram_tensor("Wlin", [HID, cfg.num_classes], F16, kind="ExternalInput")
    brow1_d = nc.dram_tensor("brow1", [1, HID], F16, kind="ExternalInput")
    brow2_d = nc.dram_tensor("brow2", [1, HID], F16, kind="ExternalInput")
    brow3_d = nc.dram_tensor("brow3", [1, HID], F16, kind="ExternalInput")
    blinrow_d = nc.dram_tensor("blinrow", [1, cfg.num_classes], F16, kind="ExternalInput")
    onesrow_d = nc.dram_tensor("onesrow", [1, GPC], F16, kind="ExternalInput")
    out_d = nc.dram_tensor("out", [cfg.num_classes, GPC], F32, kind="ExternalOutput")

    rg = [list(range(NC))]

    with tile.TileContext(nc) as tc, ExitStack() as ctx:
        sb = ctx.enter_context(tc.tile_pool(name="sb", bufs=1))
        mk = ctx.enter_context(tc.tile_pool(name="mk", bufs=2))
        hb = ctx.enter_context(tc.tile_pool(name="hb", bufs=4))
        gbuf = ctx.enter_context(tc.tile_pool(name="gbuf", bufs=2))
        ps = ctx.enter_context(tc.tile_pool(name="ps", bufs=2, space="PSUM"))
        ps2 = ctx.enter_context(tc.tile_pool(name="ps2", bufs=2, space="PSUM"))
        pool_ps = ctx.enter_context(tc.tile_pool(name="pps", bufs=1, space="PSUM"))
        dram = ctx.enter_context(tc.tile_pool(name="dram", bufs=1, space="DRAM"))

        # ---- static SBUF state ------------------------------------------
        ident16 = sb.tile([P, P], F16)
        make_identity(nc, ident16[:])

        idx_sb = sb.tile([P, NSUB * 8], I16)
        nc.sync.dma_start(idx_sb[:], idx_d[:, :])
        invdeg_sb = sb.tile([P, NT], F32)
        nc.sync.dma_start(invdeg_sb[:], invdeg_d[:, :])
        degrow_sb = sb.tile([1, cfg.n_own], F16)
        nc.sync.dma_start(degrow_sb[:], degrow_d[:, :])
        gmask_sb = sb.tile([P, NT * GPC], F16)
        nc.sync.dma_start(gmask_sb[:], gmask_d[:, :])
        degcol_sb = sb.tile([P, NT], F32)
        nc.sync.dma_start(degcol_sb[:], degcol_d[:, :])

        def load_w(d, p_, f_, nm):
            t = sb.tile([p_, f_], F16, name=nm, tag=nm)
            nc.sync.dma_start(t[:], d[:, :])
            return t

        WlWr1_sb = load_w(WlWr1_d, INF, H2, "w1s")
        WlWr2_sb = load_w(WlWr2_d, HID, H2, "w2s")
        WlWr3_sb = load_w(WlWr3_d, HID, H2, "w3s")
        Wlin_sb = load_w(Wlin_d, HID, cfg.num_classes, "wlins")
        brow1_sb = load_w(brow1_d, 1, HID, "b1s")
        brow2_sb = load_w(brow2_d, 1, HID, "b2s")
        brow3_sb = load_w(brow3_d, 1, HID, "b3s")
        blinrow_sb = load_w(blinrow_d, 1, cfg.num_classes, "bls")
        onesrow_sb = load_w(onesrow_d, 1, GPC, "o1s")

        xT_sb = sb.tile([INF, cfg.n_own], F16)
        nc.sync.dma_start(xT_sb[:], xT_d[:, :])
        for _ in range(2):
            g0 = gbuf.tile([P, GMAX * P], F16, tag="g")
            nc.vector.memset(g0[:], 0.0)
        tc.no_sync_barrier()

        # ---- internal DRAM ----------------------------------------------
        slab_agg = dram.tile([cfg.n_own, HID], F16, tag="slaba", name="slaba")
        slab_self = dram.tile([cfg.n_own, HID], F16, tag="slabs", name="slabs")
        tbls = [[dram.tile([HROWS, HID], F16, tag=f"tbl{l}{h}",
                           name=f"tbl{l}{h}", addr_space="Shared")
                 for h in range(2)] for l in range(3)]

        def ag_half(lyr, h):
            nc.gpsimd.collective_compute(
                "AllGather", mybir.AluOpType.bypass, replica_groups=rg,
                ins=[slab_agg[h * HALF:(h + 1) * HALF, :]],
                outs=[tbls[lyr][h].opt()],
            )

        def build_table_tile(t, lhsT, W_sb, first_layer):
            """M = lhs @ [Wl|Wr]; self half scaled by maxdeg; write slab rows."""
            m_ps = ps2.tile([P, H2], F32, tag="mps")
            nc.tensor.matmul(out=m_ps[:], lhsT=lhsT, rhs=W_sb[:],
                             start=True, stop=True)
            m_sb = hb.tile([P, H2], F16, tag="msb")
            # agg half: plain copy; self half: * maxdeg (per free col? no:
            # maxdeg is per NODE = per PARTITION here since m_ps is [node, :])
            nc.scalar.copy(m_sb[:, 0:HID], m_ps[:, 0:HID])
            nc.scalar.mul(m_sb[:, HID:H2], m_ps[:, HID:H2],
                          degcol_sb[:, t:t + 1])
            nc.sync.dma_start(slab_agg[t * P:(t + 1) * P, :], m_sb[:, 0:HID])
            nc.sync.dma_start(slab_self[t * P:(t + 1) * P, :], m_sb[:, HID:H2])

        # ---- P0: table1 = x @ [Wl1|Wr1] ---------------------------------
        for t in range(NT):
            build_table_tile(t, xT_sb[:, t * P:(t + 1) * P], WlWr1_sb, True)
            if t == NT // 2 - 1:
                ag_half(0, 0)
        ag_half(0, 1)

        # ---- layers ------------------------------------------------------
        for layer in range(3):
            tblh = tbls[layer]
            brow_sb = (brow1_sb, brow2_sb, brow3_sb)[layer]
            W_next = (WlWr2_sb, WlWr3_sb, None)[layer]

            if layer == 2:
                poolT_ps = pool_ps.tile([HID, GPC], F32, tag="pool")

            for b in range(NBATCH):
                g_t = gbuf.tile([P, GMAX * P], F16, tag="g")
                mk_t = mk.tile([P, GMAX * P], F16, tag="mk")
                bs0 = call_meta[b * CH][0]       # first subtile of batch
                bw = batch_width[b]
                if bw:
                    nc.sync.dma_start(
                        mk_t[:, :bw * P],
                        masks_d[:, bs0 * P:(bs0 + bw) * P])
                off_bc = []
                off = 0
                for c in range(CH):
                    s0, k_bc = call_meta[b * CH + c]
                    off_bc.append(off)
                    if k_bc == 0:
                        continue
                    nidx = k_bc * P
                    nc.gpsimd.dma_gather(
                        out_ap=g_t[:, off * P:(off + k_bc) * P].rearrange(
                            "p (t e) -> p t e", e=HID),
                        in_ap=tblh[c // CPH][
                            (c % CPH) * CHSPAN:
                            min((c % CPH + 1) * CHSPAN, HROWS), :],
                        idxs_ap=idx_sb[:, s0 * 8:(s0 + k_bc) * 8],
                        num_idxs=nidx,
                        num_idxs_reg=nidx,
                        elem_size=HID,
                        single_packet=False,
                    )
                    off += k_bc

                for ti in range(NB):
                    t = b * NB + ti
                    out_ps = ps.tile([P, HID], F32, tag="agg")
                    first = True
                    for c in range(CH):
                        base = off_bc[c]
                        # tiles before ti in this (b, c) group
                        pre = sum(KTC[(b * NB + u) * CH + c] for u in range(ti))
                        for j in range(KTC[t * CH + c]):
                            scol = sub_of[(t, c, j)]
                            gcol = base + pre + j
                            mcol = scol - bs0
                            nc.tensor.matmul(
                                out=out_ps[:],
                                lhsT=mk_t[:, mcol * P:(mcol + 1) * P],
                                rhs=g_t[:, gcol * P:(gcol + 1) * P],
                                start=first, stop=False,
                            )
                            first = False
                    # self rows: slab[t, HID:H2] = maxdeg*(h@Wr)
                    st = hb.tile([P, HID], F16, tag="st")
                    nc.sync.dma_start(st[:], slab_self[t * P:(t + 1) * P, :])
                    nc.tensor.matmul(
                        out=out_ps[:], lhsT=ident16[:], rhs=st[:],
                        start=first, stop=False,
                    )
                    # bias: += maxdeg[n] * b[f] (rank-1)
                    nc.tensor.matmul(
                        out=out_ps[:],
                        lhsT=degrow_sb[:, t * P:(t + 1) * P],
                        rhs=brow_sb[:],
                        start=False, stop=True,
                    )

                    h_sb = hb.tile([P, HID], F16, tag="h")
                    nc.scalar.activation(
                        h_sb[:], out_ps[:],
                        (mybir.ActivationFunctionType.Relu if layer < 2
                         else mybir.ActivationFunctionType.Identity),
                        bias=0.0,
                        scale=invdeg_sb[:, t:t + 1],
                    )

                    if layer < 2:
                        # hT via PE transpose, then table build
                        hT_ps = ps2.tile([P, HID], F16, tag="htps")
                        nc.tensor.transpose(hT_ps[:], h_sb[:], ident16[:])
                        hT_sb = hb.tile([P, HID], F16, tag="htsb")
                        nc.scalar.copy(hT_sb[:], hT_ps[:])
                        build_table_tile(t, hT_sb[:], W_next, False)
                    else:
                        nc.tensor.matmul(
                            out=poolT_ps[:], lhsT=h_sb[:],
                            rhs=gmask_sb[:, t * GPC:(t + 1) * GPC],
                            start=(t == 0), stop=(t == NT - 1),
                        )
                if layer < 2 and b == AGA_BATCH:
                    ag_half(layer + 1, 0)
                tc.no_sync_barrier()

            if layer < 2:
                ag_half(layer + 1, 1)

        # ---- head --------------------------------------------------------
        poolT_sb = sb.tile([HID, GPC], F16)
        nc.vector.tensor_copy(poolT_sb[:], poolT_ps[:])
        fin_ps = pool_ps.tile([cfg.num_classes, GPC], F32, tag="fin")
        nc.tensor.matmul(
            out=fin_ps[:], lhsT=Wlin_sb[:], rhs=poolT_sb[:],
            start=True, stop=False,
        )
        nc.tensor.matmul(
            out=fin_ps[:], lhsT=blinrow_sb[:], rhs=onesrow_sb[:],
            start=False, stop=True,
        )
        fin_sb = sb.tile([cfg.num_classes, GPC], F32)
        nc.vector.tensor_copy(fin_sb[:], fin_ps[:])
        nc.sync.dma_start(out_d[:, :], fin_sb[:])

    nc.compile()
    return nc


# revision 6
# speedup vs baseline: 1.4144x; 1.0359x over previous
"""Distributed GraphSAGE kernel for Trainium2 (8 NeuronCores, Bass/Tile) — v2.

Row-major aggregation scheme:
  - table row v (512B fp16): [ h_v @ Wl_next (128) | maxdeg_v * (h_v @ Wr_next) (128) ]
  - gather (SWDGE dma_gather, elem 256B, elem_step 512B) fetches only the agg
    half of m[src] for each edge slot; exact per-(tile,chunk) subtile counts
  - per dst tile: PSUM[dst,f] += onehot_mask(lhsT) @ g(rhs) over subtiles,
    += ident @ self_rows (slab-local), += maxdeg-row (x) bias-row (rank-1)
  - ACT: h = relu(PSUM * (1/maxdeg))  (per-partition scale = per-dst-node)
  - table build: hT via PE transpose, M = hT^T @ [Wl|Wr], self half scaled by
    maxdeg during the PSUM->SBUF fp16 copy, slab written row-major, AllGather
    (Shared output) -> next table
  - layer 3: h3 = PSUM * invdeg (identity act), pooling via host-shipped
    per-tile graph one-hot/gsize masks: poolT[f,G] += h3(lhsT) @ gmask(rhs)
"""
import sys

sys.path.insert(0, "/opt/trn_rl_repo")

import numpy as np
from contextlib import ExitStack
from dataclasses import dataclass

from concourse import bass, mybir, tile, bacc
from concourse import bass_utils
from concourse.masks import make_identity

P = 128
CHSPAN = 32768      # table rows per gather chunk (int16 index limit)
F16 = mybir.dt.float16
F32 = mybir.dt.float32
I16 = mybir.dt.int16

DST_SENTINEL = -300.0
MASK_ENGINE = "split"   # 'dve' | 'act' | 'split'


@dataclass(frozen=True)
class Cfg:
    n_cores: int
    num_nodes: int
    num_edges: int
    in_feat: int
    hidden: int
    num_graphs: int
    num_classes: int
    n_own: int              # padded nodes per core (multiple of NB*128)
    nb: int                 # node tiles per gather batch
    gpc: int                # graphs per core
    k_tc: tuple             # per (tile, chunk) subtile counts, len NT*CH

    @property
    def nt(self):
        return self.n_own // P

    @property
    def half(self):
        return self.n_own // 2

    @property
    def ch_per_half(self):
        return -(-(self.n_cores * self.half) // CHSPAN)

    @property
    def ch(self):
        return 2 * self.ch_per_half

    @property
    def nsub(self):
        return sum(self.k_tc)


def build_program(cfg: Cfg):
    nc = bacc.Bacc(
        "TRN2",
        target_bir_lowering=False,
        debug=False,
        num_devices=cfg.n_cores,
        num_swdge_queues=1,
    )

    NT, NB = cfg.nt, cfg.nb
    NSUB = cfg.nsub
    HID = cfg.hidden
    H2 = 2 * HID
    INF = cfg.in_feat
    GPC = cfg.gpc
    NC = cfg.n_cores
    CH = cfg.ch
    CPH = cfg.ch_per_half
    HALF = cfg.half
    HROWS = NC * HALF
    NBATCH = NT // NB
    AGA_TILE = ((NT // 2 + NB - 1) // NB) * NB - 1   # last tile of AG_A batch
    AGA_BATCH = AGA_TILE // NB
    KTC = cfg.k_tc          # [t*CH + c]

    # derived layout: subtile order (b, c, t, j)
    # per (b,c): k_bc = sum over tiles in batch; call idx cols contiguous
    sub_of = {}             # (t, c, j) -> global subtile index
    s = 0
    call_meta = []          # per (b, c): (s0, k_bc)
    batch_width = []        # per b: total subtiles
    for b in range(NBATCH):
        w = 0
        for c in range(CH):
            s0 = s
            for ti in range(NB):
                t = b * NB + ti
                for j in range(KTC[t * CH + c]):
                    sub_of[(t, c, j)] = s
                    s += 1
            call_meta.append((s0, s - s0))
            w += s - s0
        batch_width.append(w)
    assert s == NSUB
    GMAX = max(batch_width) if batch_width else 1

    # ---- I/O -------------------------------------------------------------
    xT_d = nc.dram_tensor("xT", [INF, cfg.n_own], F16, kind="ExternalInput")
    idx_d = nc.dram_tensor("eidx", [P, NSUB * 8], I16, kind="ExternalInput")
    masks_d = nc.dram_tensor("emasks", [P, NSUB * P], F16, kind="ExternalInput")
    invdeg_d = nc.dram_tensor("invdeg", [P, NT], F32, kind="ExternalInput")
    degrow_d = nc.dram_tensor("degrow", [1, cfg.n_own], F16, kind="ExternalInput")
    gmask_d = nc.dram_tensor("gmask", [P, NT * GPC], F16, kind="ExternalInput")
    degcol_d = nc.dram_tensor("degcol", [P, NT], F32, kind="ExternalInput")
    WlWr1_d = nc.dram_tensor("WlWr1", [INF, H2], F16, kind="ExternalInput")
    WlWr2_d = nc.dram_tensor("WlWr2", [HID, H2], F16, kind="ExternalInput")
    WlWr3_d = nc.dram_tensor("WlWr3", [HID, H2], F16, kind="ExternalInput")
    Wlin_d = nc.dram_tensor("Wlin", [HID, cfg.num_classes], F16, kind="ExternalInput")
    brow1_d = nc.dram_tensor("brow1", [1, HID], F16, kind="ExternalInput")
    brow2_d = nc.dram_tensor("brow2", [1, HID], F16, kind="ExternalInput")
    brow3_d = nc.dram_tensor("brow3", [1, HID], F16, kind="ExternalInput")
    blinrow_d = nc.dram_tensor("blinrow", [1, cfg.num_classes], F16, kind="ExternalInput")
    onesrow_d = nc.dram_tensor("onesrow", [1, GPC], F16, kind="ExternalInput")
    out_d = nc.dram_tensor("out", [cfg.num_classes, GPC], F32, kind="ExternalOutput")

    rg = [list(range(NC))]

    with tile.TileContext(nc) as tc, ExitStack() as ctx:
        sb = ctx.enter_context(tc.tile_pool(name="sb", bufs=1))
        mk = ctx.enter_context(tc.tile_pool(name="mk", bufs=2))
        hb = ctx.enter_context(tc.tile_pool(name="hb", bufs=4))
        gbuf = ctx.enter_context(tc.tile_pool(name="gbuf", bufs=7))
        ps = ctx.enter_context(tc.tile_pool(name="ps", bufs=2, space="PSUM"))
        ps2 = ctx.enter_context(tc.tile_pool(name="ps2", bufs=2, space="PSUM"))
        pool_ps = ctx.enter_context(tc.tile_pool(name="pps", bufs=1, space="PSUM"))
        dram = ctx.enter_context(tc.tile_pool(name="dram", bufs=1, space="DRAM"))

        # ---- static SBUF state ------------------------------------------
        ident16 = sb.tile([P, P], F16)
        make_identity(nc, ident16[:])

        idx_sb = sb.tile([P, NSUB * 8], I16)
        nc.sync.dma_start(idx_sb[:], idx_d[:, :])
        invdeg_sb = sb.tile([P, NT], F32)
        nc.sync.dma_start(invdeg_sb[:], invdeg_d[:, :])
        degrow_sb = sb.tile([1, cfg.n_own], F16)
        nc.sync.dma_start(degrow_sb[:], degrow_d[:, :])
        gmask_sb = sb.tile([P, NT * GPC], F16)
        nc.sync.dma_start(gmask_sb[:], gmask_d[:, :])
        degcol_sb = sb.tile([P, NT], F32)
        nc.sync.dma_start(degcol_sb[:], degcol_d[:, :])

        def load_w(d, p_, f_, nm):
            t = sb.tile([p_, f_], F16, name=nm, tag=nm)
            nc.sync.dma_start(t[:], d[:, :])
            return t

        WlWr1_sb = load_w(WlWr1_d, INF, H2, "w1s")
        WlWr2_sb = load_w(WlWr2_d, HID, H2, "w2s")
        WlWr3_sb = load_w(WlWr3_d, HID, H2, "w3s")
        Wlin_sb = load_w(Wlin_d, HID, cfg.num_classes, "wlins")
        brow1_sb = load_w(brow1_d, 1, HID, "b1s")
        brow2_sb = load_w(brow2_d, 1, HID, "b2s")
        brow3_sb = load_w(brow3_d, 1, HID, "b3s")
        blinrow_sb = load_w(blinrow_d, 1, cfg.num_classes, "bls")
        onesrow_sb = load_w(onesrow_d, 1, GPC, "o1s")

        xT_sb = sb.tile([INF, cfg.n_own], F16)
        nc.sync.dma_start(xT_sb[:], xT_d[:, :])
        for _ in range(7):
            g0 = gbuf.tile([P, GMAX * P], F16, tag="g")
            nc.vector.memset(g0[:], 0.0)
        tc.no_sync_barrier()

        # ---- internal DRAM ----------------------------------------------
        slab_agg = dram.tile([cfg.n_own, HID], F16, tag="slaba", name="slaba")
        slab_self = dram.tile([cfg.n_own, HID], F16, tag="slabs", name="slabs")
        tbls = [[dram.tile([HROWS, HID], F16, tag=f"tbl{l}{h}",
                           name=f"tbl{l}{h}", addr_space="Shared")
                 for h in range(2)] for l in range(3)]

        def ag_half(lyr, h):
            nc.gpsimd.collective_compute(
                "AllGather", mybir.AluOpType.bypass, replica_groups=rg,
                ins=[slab_agg[h * HALF:(h + 1) * HALF, :]],
                outs=[tbls[lyr][h].opt()],
            )

        def build_table_tile(t, lhsT, W_sb, first_layer):
            """M = lhs @ [Wl|Wr]; self half scaled by maxdeg; write slab rows."""
            m_ps = ps2.tile([P, H2], F32, tag="mps")
            nc.tensor.matmul(out=m_ps[:], lhsT=lhsT, rhs=W_sb[:],
                             start=True, stop=True)
            m_sb = hb.tile([P, H2], F16, tag="msb")
            # agg half: plain copy; self half: * maxdeg (per free col? no:
            # maxdeg is per NODE = per PARTITION here since m_ps is [node, :])
            nc.scalar.copy(m_sb[:, 0:HID], m_ps[:, 0:HID])
            nc.scalar.mul(m_sb[:, HID:H2], m_ps[:, HID:H2],
                          degcol_sb[:, t:t + 1])
            nc.sync.dma_start(slab_agg[t * P:(t + 1) * P, :], m_sb[:, 0:HID])
            nc.sync.dma_start(slab_self[t * P:(t + 1) * P, :], m_sb[:, HID:H2])

        # ---- P0: table1 = x @ [Wl1|Wr1] ---------------------------------
        for t in range(NT):
            build_table_tile(t, xT_sb[:, t * P:(t + 1) * P], WlWr1_sb, True)
            if t == NT // 2 - 1:
                ag_half(0, 0)
        ag_half(0, 1)

        # ---- layers ------------------------------------------------------
        for layer in range(3):
            tblh = tbls[layer]
            brow_sb = (brow1_sb, brow2_sb, brow3_sb)[layer]
            W_next = (WlWr2_sb, WlWr3_sb, None)[layer]

            if layer == 2:
                poolT_ps = pool_ps.tile([HID, GPC], F32, tag="pool")

            HOIST = min(4, NBATCH)

            def emit_gathers(b, g_t, crange, off0):
                off = off0
                offs = []
                for c in crange:
                    s0, k_bc = call_meta[b * CH + c]
                    offs.append(off)
                    if k_bc == 0:
                        continue
                    nidx = k_bc * P
                    nc.gpsimd.dma_gather(
                        out_ap=g_t[:, off * P:(off + k_bc) * P].rearrange(
                            "p (t e) -> p t e", e=HID),
                        in_ap=tblh[c // CPH][
                            (c % CPH) * CHSPAN:
                            min((c % CPH + 1) * CHSPAN, HROWS), :],
                        idxs_ap=idx_sb[:, s0 * 8:(s0 + k_bc) * 8],
                        num_idxs=nidx,
                        num_idxs_reg=nidx,
                        elem_size=HID,
                        single_packet=False,
                    )
                    off += k_bc
                return offs, off

            g_pend = {}
            for hb_ in range(HOIST):
                g_t = gbuf.tile([P, GMAX * P], F16, tag="g")
                offs, off = emit_gathers(hb_, g_t, range(CPH), 0)
                g_pend[hb_] = (g_t, offs, off)

            for b in range(NBATCH):
                if b in g_pend:
                    g_t, offs_lo, off = g_pend.pop(b)
                else:
                    g_t = gbuf.tile([P, GMAX * P], F16, tag="g")
                    offs_lo, off = emit_gathers(b, g_t, range(CPH), 0)
                offs_hi, off = emit_gathers(b, g_t, range(CPH, CH), off)
                off_bc = offs_lo + offs_hi
                mk_t = mk.tile([P, GMAX * P], F16, tag="mk")
                bs0 = call_meta[b * CH][0]       # first subtile of batch
                bw = batch_width[b]
                if bw:
                    nc.sync.dma_start(
                        mk_t[:, :bw * P],
                        masks_d[:, bs0 * P:(bs0 + bw) * P])

                for ti in range(NB):
                    t = b * NB + ti
                    out_ps = ps.tile([P, HID], F32, tag="agg")
                    first = True
                    for c in range(CH):
                        base = off_bc[c]
                        # tiles before ti in this (b, c) group
                        pre = sum(KTC[(b * NB + u) * CH + c] for u in range(ti))
                        for j in range(KTC[t * CH + c]):
                            scol = sub_of[(t, c, j)]
                            gcol = base + pre + j
                            mcol = scol - bs0
                            nc.tensor.matmul(
                                out=out_ps[:],
                                lhsT=mk_t[:, mcol * P:(mcol + 1) * P],
                                rhs=g_t[:, gcol * P:(gcol + 1) * P],
                                start=first, stop=False,
                            )
                            first = False
                    # self rows: slab[t, HID:H2] = maxdeg*(h@Wr)
                    st = hb.tile([P, HID], F16, tag="st")
                    nc.sync.dma_start(st[:], slab_self[t * P:(t + 1) * P, :])
                    nc.tensor.matmul(
                        out=out_ps[:], lhsT=ident16[:], rhs=st[:],
                        start=first, stop=False,
                    )
                    # bias: += maxdeg[n] * b[f] (rank-1)
                    nc.tensor.matmul(
                        out=out_ps[:],
                        lhsT=degrow_sb[:, t * P:(t + 1) * P],
                        rhs=brow_sb[:],
                        start=False, stop=True,
                    )

                    h_sb = hb.tile([P, HID], F16, tag="h")
                    nc.scalar.activation(
                        h_sb[:], out_ps[:],
                        (mybir.ActivationFunctionType.Relu if layer < 2
                         else mybir.ActivationFunctionType.Identity),
                        bias=0.0,
                        scale=invdeg_sb[:, t:t + 1],
                    )

                    if layer < 2:
                        # hT via PE transpose, then table build
                        hT_ps = ps2.tile([P, HID], F16, tag="htps")
                        nc.tensor.transpose(hT_ps[:], h_sb[:], ident16[:])
                        hT_sb = hb.tile([P, HID], F16, tag="htsb")
                        nc.scalar.copy(hT_sb[:], hT_ps[:])
                        build_table_tile(t, hT_sb[:], W_next, False)
                    else:
                        nc.tensor.matmul(
                            out=poolT_ps[:], lhsT=h_sb[:],
                            rhs=gmask_sb[:, t * GPC:(t + 1) * GPC],
                            start=(t == 0), stop=(t == NT - 1),
                        )
                if layer < 2 and b == AGA_BATCH:
                    ag_half(layer + 1, 0)
                tc.no_sync_barrier()

            if layer < 2:
                ag_half(layer + 1, 1)

        # ---- head --------------------------------------------------------
        poolT_sb = sb.tile([HID, GPC], F16)
        nc.vector.tensor_copy(poolT_sb[:], poolT_ps[:])
        fin_ps = pool_ps.tile([cfg.num_classes, GPC], F32, tag="fin")
        nc.tensor.matmul(
            out=fin_ps[:], lhsT=Wlin_sb[:], rhs=poolT_sb[:],
            start=True, stop=False,
        )
        nc.tensor.matmul(
            out=fin_ps[:], lhsT=blinrow_sb[:], rhs=onesrow_sb[:],
            start=False, stop=True,
        )
        fin_sb = sb.tile([cfg.num_classes, GPC], F32)
        nc.vector.tensor_copy(fin_sb[:], fin_ps[:])
        nc.sync.dma_start(out_d[:, :], fin_sb[:])

    nc.compile()
    return nc
